# revision 1
# baseline (speedup 1.0000x reference)
"""Trainium2 Bass kernel for the 3-metalayer forward-forward style MLP.

Distribution: the (10 labels x 512 batch) grid flattens to 5120 independent
rows; each of the 8 cores processes 640 rows (pure data parallelism, weights
replicated, no collectives).

Device-side algorithm (per core, rows R=640):
  - states kept feature-major [2048(part-chunks), R] in bf16, pre-normalized
  - per linear term: 16x2x16 PE matmuls (128x128 lhsT weight tiles, N=320),
    fp32 PSUM accumulate, ACT relu+bias eviction
  - 0.7/0.3 metalayer blend folded into host-prescaled weights/biases
    (relu positive homogeneity)
  - row L2 norms: square (ACT) + ones-vector PE matmul reduction over
    partitions; 1/(sqrt+eps) on DVE; broadcast back over partitions with a
    K=1 PE matmul; goodness = sum(s^2)/2048 falls out of the same machinery
  - t=0 terms with zero-state inputs are host-folded constants; the layer-1
    "pre" term (static overlay input) is computed once and reused all 3 steps
"""

import numpy as np
import ml_dtypes

import concourse.bass as bass
import concourse.tile as tile
from concourse import bacc, mybir
from concourse.bass_utils import run_bass_kernel_spmd

BF = mybir.dt.bfloat16
F32 = mybir.dt.float32
NPBF = ml_dtypes.bfloat16

N_CORES = 8
P = 128
D_IN = 784
D_IN_PAD = 896            # 7 * 128
KC1 = 7                   # k-chunks for the 784->2048 matmul
KC = 16                   # k-chunks for 2048-contraction matmuls
MC = 16                   # output-feature chunks (2048 / 128)
H = 2048
B = 512
NL = 10
ROWS = NL * B             # 5120
R = ROWS // N_CORES       # 640 rows per core
RH = 320                  # psum row-chunk (2 per core-row-block)
EPS = 1e-4

# bias/const column indices inside the packed [128, 12*16] bias tensor
B1PRE, B1POST, B1SELF, B2PRE, B2POST, B2SELF, B3PRE, B3SELF, C1, C2, C3, C3P = range(12)
NBIAS = 12

_NC_CACHE = {}


def _build_nc():
    """Build the single-core Tile program (same NEFF for all 8 cores)."""
    nc = bacc.Bacc("TRN2", target_bir_lowering=False, debug=False,
                   num_devices=N_CORES)

    hx_d = nc.dram_tensor("hxn", [P, KC1, R], BF, kind="ExternalInput")
    w_d = {
        "w1pre": nc.dram_tensor("w1pre", [MC, P, KC1, P], BF, kind="ExternalInput"),
    }
    for name in ("w1post", "w1self", "w2pre", "w2post", "w2self", "w3pre", "w3self"):
        w_d[name] = nc.dram_tensor(name, [MC, P, KC, P], BF, kind="ExternalInput")
    bias_d = nc.dram_tensor("biases", [P, NBIAS * MC], F32, kind="ExternalInput")
    g_d = nc.dram_tensor("g", [1, R], F32, kind="ExternalOutput")

    with tile.TileContext(nc) as tc:
        with (
            tc.tile_pool(name="consts", bufs=1) as consts,
            tc.tile_pool(name="states", bufs=1) as states,
            tc.tile_pool(name="wpool", bufs=8) as wpool,
            tc.tile_pool(name="epool", bufs=6) as epool,
            tc.tile_pool(name="sqpool", bufs=6) as sqpool,
            tc.tile_pool(name="small", bufs=2) as small,
            tc.tile_pool(name="mmps", bufs=6, space="PSUM") as mmps,
            tc.tile_pool(name="redps", bufs=2, space="PSUM") as redps,
        ):
            # startup order: first hx chunk + first weight block must land
            # before anything else so the PE starts within ~1.5us
            hx = states.tile([P, KC1, R], BF, tag="hxn")
            nc.sync.dma_start(out=hx[:, 0, :], in_=hx_d[:, 0, :])
            bias_sb = consts.tile([P, NBIAS * MC], F32)
            w0 = wpool.tile([P, KC1, P], BF, tag="w", name="w1pre0")
            nc.sync.dma_start(out=w0[:], in_=w_d["w1pre"][0])
            nc.sync.dma_start(out=bias_sb[:], in_=bias_d[:])
            for kc in range(1, KC1):
                nc.sync.dma_start(out=hx[:, kc, :], in_=hx_d[:, kc, :])
            # [128, 128] ones: M=128 ones-matmul both reduces over partitions
            # AND broadcasts the row sum-of-squares to every partition for free
            ones_red = consts.tile([P, P], BF)
            nc.vector.memset(ones_red[:], 1.0)
            gacc = consts.tile([1, R], F32)

            # warm the PE HAM clock gate while the initial DMAs are in
            # flight: ~25 dummy matmuls span >3.4us of PE activity, so the
            # real matmul stream starts at 2.4GHz instead of 1.2GHz
            warm_ps = mmps.tile([P, RH], F32, tag="mm", name="warm_ps")
            for _ in range(64):
                nc.tensor.matmul(warm_ps[:, :P], ones_red[:], ones_red[:],
                                 start=True, stop=True)
            At = states.tile([P, MC, R], BF, tag="A")
            s1 = states.tile([P, MC, R], BF, tag="s1")
            s2 = states.tile([P, MC, R], BF, tag="s2")
            s3 = states.tile([P, MC, R], BF, tag="s3")
            snew = states.tile([P, MC, R], BF, tag="snew")
            comb = states.tile([P, MC, R], BF, tag="comb")

            _red_uid = [0]

            def red_pair():
                _red_uid[0] += 1
                u = _red_uid[0]
                return (redps.tile([P, RH], F32, tag="red", name=f"red{u}a"),
                        redps.tile([P, RH], F32, tag="red", name=f"red{u}b"))

            def bias_ap(idx, mc):
                col = idx * MC + mc
                return bias_sb[:, col:col + 1]

            def rsl(rh):
                return slice(rh * RH, (rh + 1) * RH)

            def term_pass(wname, kcn, src, evict, w0_tile=None, defer=2):
                """One linear term: stream weight blocks, accumulate psums,
                hand each [128, RH] psum chunk to `evict(mc, rh, ps)`.

                Evictions are emitted `defer` psum-groups late: the eviction
                chain (ACT relu -> DVE combine/square -> PE reduce-matmul)
                has ~1.5us of cross-engine latency, and emitting it inline
                makes the strict-FIFO PE queue stall on the reduce-matmul.
                Deferring places it behind independent matmul work."""
                wd = w_d[wname]
                pending = []
                for mc in range(MC):
                    if mc == 0 and w0_tile is not None:
                        wt = w0_tile
                    else:
                        wt = wpool.tile([P, kcn, P], BF, tag="w")
                        nc.sync.dma_start(out=wt[:], in_=wd[mc])
                    for rh in range(2):
                        ps = mmps.tile([P, RH], F32, tag="mm")
                        for kc in range(kcn):
                            nc.tensor.matmul(
                                ps[:], wt[:, kc, :], src[:, kc, rsl(rh)],
                                start=(kc == 0), stop=(kc == kcn - 1))
                        pending.append((mc, rh, ps))
                        if len(pending) > defer:
                            evict(*pending.pop(0))
                while pending:
                    evict(*pending.pop(0))

            def sq_and_reduce(mc, rh, red):
                """Square the freshly written snew chunk; accumulate row
                sum-of-squares into the red psum via a ones-matmul."""
                sq = sqpool.tile([P, RH], BF, tag="sq")
                # on DVE (not ACT): keeps the ACT queue pure relu-evictions,
                # avoiding head-of-line blocking behind the DVE combine
                nc.vector.tensor_mul(sq[:], snew[:, mc, rsl(rh)],
                                     snew[:, mc, rsl(rh)])
                nc.tensor.matmul(red[rh][:], ones_red[:], sq[:],
                                 start=(mc == 0), stop=(mc == MC - 1))

            def finale(red, tgt, goodness):
                """red[rh] holds sum(s^2) per row, already broadcast across
                all 128 partitions (M=128 ones-matmul). sqrt + eps +
                fast-reciprocal, then scale snew into tgt."""
                if goodness:
                    for rh in range(2):
                        if goodness == "init":
                            nc.vector.tensor_copy(gacc[:, rsl(rh)],
                                                  red[rh][0:1, :])
                        else:
                            nc.vector.tensor_add(gacc[:, rsl(rh)],
                                                 gacc[:, rsl(rh)],
                                                 red[rh][0:1, :])
                if tgt is None:
                    return
                nr = small.tile([P, R], F32, tag="nr")
                for rh in range(2):
                    nc.scalar.sqrt(nr[:, rsl(rh)], red[rh][:])
                nc.vector.tensor_scalar_add(nr[:], nr[:], EPS)
                inv = small.tile([P, R], F32, tag="inv")
                nc.vector.reciprocal_approx_fast(out=inv[:], in_=nr[:])
                for rh in range(2):
                    for mc in range(MC):
                        nc.vector.tensor_mul(tgt[:, mc, rsl(rh)],
                                             snew[:, mc, rsl(rh)],
                                             inv[:, rsl(rh)])

            def evict_to(dst, bidx):
                def ev(mc, rh, ps):
                    nc.scalar.activation(
                        dst[:, mc, rsl(rh)], ps[:],
                        mybir.ActivationFunctionType.Relu,
                        bias=bias_ap(bidx, mc))
                return ev

            def evict_add_comb(bidx):
                def ev(mc, rh, ps):
                    e = epool.tile([P, RH], F32, tag="e")
                    nc.scalar.activation(
                        e[:], ps[:], mybir.ActivationFunctionType.Relu,
                        bias=bias_ap(bidx, mc))
                    nc.vector.tensor_add(comb[:, mc, rsl(rh)],
                                         e[:], comb[:, mc, rsl(rh)])
                return ev

            # ---- A = relu(hxn @ w1pre' + 0.7*b1pre), cached for all steps.
            # t0-n1 (snew = A + c1) is fused into the same pass so its
            # elementwise work overlaps the A matmuls chunk by chunk.
            red = red_pair()

            def ev_a(mc, rh, ps, red=red):
                nc.scalar.activation(
                    At[:, mc, rsl(rh)], ps[:],
                    mybir.ActivationFunctionType.Relu,
                    bias=bias_ap(B1PRE, mc))
                nc.vector.tensor_scalar_add(
                    snew[:, mc, rsl(rh)], At[:, mc, rsl(rh)],
                    bias_ap(C1, mc))
                sq_and_reduce(mc, rh, red)

            # defer=4: the A pass produces chunks every ~0.95us (7 k-chunks),
            # so the ~1.5us eviction chain needs extra slack to stay hidden
            term_pass("w1pre", KC1, hx, ev_a, w0_tile=w0, defer=4)
            finale(red, s1, None)

            # ---- t0, n2 / n3: single pre-term + const.
            # t1-n1's post/self term passes are wedged between them: they
            # only need s2(t0)/s1(t0) and don't touch comb (the t0 updates
            # don't use it), so their matmuls fill t0's serial-chain tails.
            def ev_t0(red, cidx, bpre):
                def ev(mc, rh, ps):
                    e = epool.tile([P, RH], F32, tag="e")
                    nc.scalar.activation(
                        e[:], ps[:], mybir.ActivationFunctionType.Relu,
                        bias=bias_ap(bpre, mc))
                    nc.vector.tensor_scalar_add(
                        snew[:, mc, rsl(rh)], e[:], bias_ap(cidx, mc))
                    sq_and_reduce(mc, rh, red)
                return ev

            red = red_pair()
            term_pass("w2pre", KC, s1, ev_t0(red, C2, B2PRE))
            finale(red, s2, None)

            term_pass("w1post", KC, s2, evict_to(comb, B1POST))
            term_pass("w1self", KC, s1, evict_add_comb(B1SELF))

            red = red_pair()
            term_pass("w3pre", KC, s2, ev_t0(red, C3, B3PRE))
            finale(red, s3, None)

            def n1_combine(last):
                red = red_pair()
                for mc in range(MC):
                    for rh in range(2):
                        nc.vector.tensor_add(snew[:, mc, rsl(rh)],
                                             At[:, mc, rsl(rh)],
                                             comb[:, mc, rsl(rh)])
                        sq_and_reduce(mc, rh, red)
                finale(red, s1, "init" if last else None)

            # ---- t1 / t2
            for t in (1, 2):
                last = (t == 2)
                # n1 = A + relu(s2@w1post'+b) + relu(s1@w1self'+b)
                if t == 2:
                    term_pass("w1post", KC, s2, evict_to(comb, B1POST))
                    term_pass("w1self", KC, s1, evict_add_comb(B1SELF))
                n1_combine(last)

                # n2 = relu(s1new@w2pre') + relu(s3@w2post') + relu(s2@w2self')
                term_pass("w2post", KC, s3, evict_to(comb, B2POST))
                term_pass("w2self", KC, s2, evict_add_comb(B2SELF))
                red = red_pair()

                def ev_n2(mc, rh, ps, red=red):
                    e = epool.tile([P, RH], F32, tag="e")
                    nc.scalar.activation(
                        e[:], ps[:], mybir.ActivationFunctionType.Relu,
                        bias=bias_ap(B2PRE, mc))
                    nc.vector.tensor_add(snew[:, mc, rsl(rh)],
                                         e[:], comb[:, mc, rsl(rh)])
                    sq_and_reduce(mc, rh, red)

                term_pass("w2pre", KC, s1, ev_n2)
                finale(red, s2, "add" if last else None)

                # n3 = relu(s2new@w3pre') + c3p + relu(s3@w3self')
                term_pass("w3self", KC, s3, evict_to(comb, B3SELF))
                red = red_pair()

                def ev_n3(mc, rh, ps, red=red):
                    e = epool.tile([P, RH], F32, tag="e")
                    nc.scalar.activation(
                        e[:], ps[:], mybir.ActivationFunctionType.Relu,
                        bias=bias_ap(B3PRE, mc))
                    nc.vector.scalar_tensor_tensor(
                        snew[:, mc, rsl(rh)], e[:], bias_ap(C3P, mc),
                        comb[:, mc, rsl(rh)],
                        op0=mybir.AluOpType.add, op1=mybir.AluOpType.add)
                    sq_and_reduce(mc, rh, red)

                term_pass("w3pre", KC, s2, ev_n3)
                finale(red, None if last else s3, "add" if last else None)

            # ---- goodness out: g = gacc / 2048
            gout = consts.tile([1, R], F32, tag="gout")
            nc.scalar.mul(gout[:], gacc[:], 1.0 / H)
            nc.sync.dma_start(out=g_d[:], in_=gout[:])

    nc.compile()
    return nc


def _block_weight(w, scale, kcn):
    """[2048, d_in] float32 -> [MC, P, kcn, P] bf16 blocked for linear DMA:
    host_w[mc, p, kc, m] = scale * W[mc*128+m, kc*128+p]."""
    w = np.asarray(w, dtype=np.float32) * scale
    din = w.shape[1]
    if din < kcn * P:
        w = np.pad(w, ((0, 0), (0, kcn * P - din)))
    blk = w.reshape(MC, P, kcn, P).transpose(0, 3, 2, 1)
    return np.ascontiguousarray(blk.astype(NPBF))


def _col(v):
    """[2048] -> [128, 16] (partition-major bias layout)."""
    return np.asarray(v, dtype=np.float32).reshape(MC, P).T


def prepare_inputs(inputs):
    """Host prep: overlay+normalize Hx, prescale/block weights, pack biases.
    Returns (shared_map, per_core_hx list)."""
    x = np.asarray(inputs["x"], dtype=np.float32)
    mx = x.max()
    base = x.copy()
    base[:, :NL] = 0.0
    hx = np.tile(base[None, :, :], (NL, 1, 1))
    for l in range(NL):
        hx[l, :, l] = mx
    hx = hx.reshape(ROWS, D_IN)
    n = np.linalg.norm(hx, axis=1, keepdims=True)
    hxn = hx / (n + EPS)
    hxn = np.pad(hxn, ((0, 0), (0, D_IN_PAD - D_IN)))

    per_core_hx = []
    for c in range(N_CORES):
        h = hxn[c * R:(c + 1) * R].T            # [896, 640]
        h = h.reshape(KC1, P, R).transpose(1, 0, 2)
        per_core_hx.append(np.ascontiguousarray(h.astype(NPBF)))

    shared = {
        "w1pre": _block_weight(inputs["w1_pre"], 0.7, KC1),
        "w1post": _block_weight(inputs["w1_post"], 0.7, KC),
        "w1self": _block_weight(inputs["w1_self"], 0.3, KC),
        "w2pre": _block_weight(inputs["w2_pre"], 0.7, KC),
        "w2post": _block_weight(inputs["w2_post"], 0.7, KC),
        "w2self": _block_weight(inputs["w2_self"], 0.3, KC),
        "w3pre": _block_weight(inputs["w3_pre"], 0.7, KC),
        "w3self": _block_weight(inputs["w3_self"], 0.3, KC),
    }

    relu = lambda a: np.maximum(np.asarray(a, dtype=np.float32), 0.0)

    cols = np.empty((P, NBIAS * MC), dtype=np.float32)
    vals = {
        B1PRE: 0.7 * np.asarray(inputs["b1_pre"], np.float32),
        B1POST: 0.7 * np.asarray(inputs["b1_post"], np.float32),
        B1SELF: 0.3 * np.asarray(inputs["b1_self"], np.float32),
        B2PRE: 0.7 * np.asarray(inputs["b2_pre"], np.float32),
        B2POST: 0.7 * np.asarray(inputs["b2_post"], np.float32),
        B2SELF: 0.3 * np.asarray(inputs["b2_self"], np.float32),
        B3PRE: 0.7 * np.asarray(inputs["b3_pre"], np.float32),
        B3SELF: 0.3 * np.asarray(inputs["b3_self"], np.float32),
        C1: 0.7 * relu(inputs["b1_post"]) + 0.3 * relu(inputs["b1_self"]),
        C2: 0.7 * relu(inputs["b2_post"]) + 0.3 * relu(inputs["b2_self"]),
        C3: 0.7 * relu(inputs["b3_post"]) + 0.3 * relu(inputs["b3_self"]),
        C3P: 0.7 * relu(inputs["b3_post"]),
    }
    for idx, v in vals.items():
        cols[:, idx * MC:(idx + 1) * MC] = _col(v)
    shared["biases"] = np.ascontiguousarray(cols)

    return shared, per_core_hx


def run(inputs, trace=False):
    shared, per_core_hx = prepare_inputs(inputs)
    if "nc" not in _NC_CACHE:
        _NC_CACHE["nc"] = _build_nc()
    nc = _NC_CACHE["nc"]
    in_maps = [dict(shared, hxn=per_core_hx[c]) for c in range(N_CORES)]
    res = run_bass_kernel_spmd(nc, in_maps, core_ids=list(range(N_CORES)),
                               trace=trace)
    g = np.concatenate([res.results[c]["g"][0] for c in range(N_CORES)])
    out = g.reshape(NL, B).T.astype(np.float32)
    return np.ascontiguousarray(out), res


def kernel(**inputs):
    out, _ = run(inputs, trace=False)
    return out



# revision 3
# speedup vs baseline: 1.8264x; 1.8264x over previous
"""Trainium2 Bass kernel for the 3-metalayer forward-forward style MLP.

Distribution: the (10 labels x 512 batch) grid flattens to 5120 independent
rows; each of the 8 cores processes 640 rows (pure data parallelism, weights
replicated, no collectives).

Device-side algorithm (per core, rows R=640):
  - matmul inputs (normalized states, overlay input, weights) are fp8 e4m3;
    every linear term runs as DoubleRow matmuls (K=256 per instruction,
    2 fp8 weights per PE cell -> 2x MACs/cycle). Scales: weights x1024,
    states x64 (relu positive homogeneity + scale-invariant row
    normalization make both exact up to the final /S^2 on goodness).
  - per linear term: 8 DoubleRow k-pair matmuls per 128-out-chunk, N=320,
    fp32 PSUM accumulate, ACT relu+bias eviction to bf16 (scale S=65536)
  - rh0/rh1 row-halves run back-to-back on the same stationary weights
  - 0.7/0.3 metalayer blend folded into host-prescaled weights/biases
  - row L2 norms: square (DVE, bf16) + ones-vector PE matmul reduction over
    partitions; G/(sqrt+S*eps) on ACT+DVE; normalized state written
    straight to fp8 by the DVE scale multiply; goodness = sum(s^2)/(2048*S^2)
  - t=0 terms with zero-state inputs are host-folded constants; the layer-1
    "pre" term (static overlay input) is computed once and reused all 3 steps
"""

import numpy as np
import ml_dtypes

import concourse.bass as bass
import concourse.tile as tile
from concourse import bacc, mybir
from concourse.bass_utils import run_bass_kernel_spmd

BF = mybir.dt.bfloat16
F32 = mybir.dt.float32
FP8 = mybir.dt.float8e4
NPBF = ml_dtypes.bfloat16
NPF8 = ml_dtypes.float8_e4m3
DR = mybir.MatmulPerfMode.DoubleRow

N_CORES = 8
P = 128
D_IN = 784
D_IN_PAD = 1024           # 8 * 128 (even k-chunk count for DoubleRow pairs)
KC1 = 8                   # k-chunks for the 784->2048 matmul (padded)
KC = 16                   # k-chunks for 2048-contraction matmuls
MC = 16                   # output-feature chunks (2048 / 128)
H = 2048
B = 512
NL = 10
ROWS = NL * B             # 5120
R = ROWS // N_CORES       # 640 rows per core
RH = 320                  # psum row-chunk (2 per core-row-block)
EPS = 1e-4

ALPHA = 1024.0            # weight fp8 gain
G = 64.0                  # normalized-state fp8 gain
S = ALPHA * G             # scale carried by evicted terms / snew / biases

# bias/const column indices inside the packed [128, 12*16] bias tensor
B1PRE, B1POST, B1SELF, B2PRE, B2POST, B2SELF, B3PRE, B3SELF, C1, C2, C3, C3P = range(12)
NBIAS = 12

_NC_CACHE = {}


def _build_nc():
    """Build the single-core Tile program (same NEFF for all 8 cores)."""
    nc = bacc.Bacc("TRN2", target_bir_lowering=False, debug=False,
                   num_devices=N_CORES)

    hx_d = nc.dram_tensor("hxn", [P, KC1, R], FP8, kind="ExternalInput")
    w_d = {
        "w1pre": nc.dram_tensor("w1pre", [MC, P, KC1, P], FP8, kind="ExternalInput"),
    }
    for name in ("w1post", "w1self", "w2pre", "w2post", "w2self", "w3pre", "w3self"):
        w_d[name] = nc.dram_tensor(name, [MC, P, KC, P], FP8, kind="ExternalInput")
    bias_d = nc.dram_tensor("biases", [P, NBIAS * MC], F32, kind="ExternalInput")
    g_d = nc.dram_tensor("g", [1, R], F32, kind="ExternalOutput")

    with tile.TileContext(nc) as tc:
        with (
            tc.tile_pool(name="consts", bufs=1) as consts,
            tc.tile_pool(name="states", bufs=1) as states,
            tc.tile_pool(name="wpool", bufs=8) as wpool,
            tc.tile_pool(name="epool", bufs=6) as epool,
            tc.tile_pool(name="sqpool", bufs=6) as sqpool,
            tc.tile_pool(name="small", bufs=2) as small,
            tc.tile_pool(name="mmps", bufs=6, space="PSUM") as mmps,
            tc.tile_pool(name="redps", bufs=2, space="PSUM") as redps,
        ):
            # startup order: first hx chunk + first weight block must land
            # before anything else so the PE starts within ~1.5us
            hx = states.tile([P, KC1, R], FP8, tag="hxn")
            nc.sync.dma_start(out=hx[:, 0:2, :], in_=hx_d[:, 0:2, :])
            bias_sb = consts.tile([P, NBIAS * MC], F32)
            w0 = wpool.tile([P, KC1, P], FP8, tag="w", name="w1pre0")
            nc.sync.dma_start(out=w0[:], in_=w_d["w1pre"][0])
            nc.sync.dma_start(out=bias_sb[:], in_=bias_d[:])
            for kc in range(2, KC1, 2):
                nc.sync.dma_start(out=hx[:, kc:kc + 2, :], in_=hx_d[:, kc:kc + 2, :])
            # [128, 128] ones: M=128 ones-matmul both reduces over partitions
            # AND broadcasts the row sum-of-squares to every partition for free
            ones_red = consts.tile([P, P], BF)
            nc.vector.memset(ones_red[:], 1.0)
            gacc = consts.tile([1, R], F32)

            # warm the PE HAM clock gate while the initial DMAs are in
            # flight: ~25 dummy matmuls span >3.4us of PE activity, so the
            # real matmul stream starts at 2.4GHz instead of 1.2GHz
            warm_ps = mmps.tile([P, RH], F32, tag="mm", name="warm_ps")
            for _ in range(64):
                nc.tensor.matmul(warm_ps[:, :P], ones_red[:], ones_red[:],
                                 start=True, stop=True)
            At = states.tile([P, MC, R], BF, tag="A")
            s1 = states.tile([P, MC, R], FP8, tag="s1")
            s2 = states.tile([P, MC, R], FP8, tag="s2")
            s3 = states.tile([P, MC, R], FP8, tag="s3")
            snew = states.tile([P, MC, R], BF, tag="snew")
            comb = states.tile([P, MC, R], BF, tag="comb")

            _red_uid = [0]

            def red_pair():
                _red_uid[0] += 1
                u = _red_uid[0]
                return (redps.tile([P, RH], F32, tag="red", name=f"red{u}a"),
                        redps.tile([P, RH], F32, tag="red", name=f"red{u}b"))

            def bias_ap(idx, mc):
                col = idx * MC + mc
                return bias_sb[:, col:col + 1]

            def rsl(rh):
                return slice(rh * RH, (rh + 1) * RH)

            def term_pass(wname, kcn, src, evict, w0_tile=None, defer=2):
                """One linear term: stream weight blocks, accumulate psums,
                hand each [128, RH] psum chunk to `evict(mc, rh, ps)`.

                DoubleRow: each matmul consumes a k-pair (K=256); the two
                row-halves run back-to-back on the same stationary weights.

                Evictions are emitted `defer` psum-groups late: the eviction
                chain (ACT relu -> DVE combine/square -> PE reduce-matmul)
                has ~1.5us of cross-engine latency, and emitting it inline
                makes the strict-FIFO PE queue stall on the reduce-matmul.
                Deferring places it behind independent matmul work."""
                wd = w_d[wname]
                kpn = kcn // 2
                pending = []
                for mc in range(MC):
                    if mc == 0 and w0_tile is not None:
                        wt = w0_tile
                    else:
                        wt = wpool.tile([P, kcn, P], FP8, tag="w")
                        nc.sync.dma_start(out=wt[:], in_=wd[mc])
                    ps = [mmps.tile([P, RH], F32, tag="mm",
                                    name=f"mm_{wname}_{mc}_{rh}")
                          for rh in range(2)]
                    for kp in range(kpn):
                        for rh in range(2):
                            nc.tensor.matmul(
                                ps[rh][:], wt[:, 2 * kp:2 * kp + 2, :],
                                src[:, 2 * kp:2 * kp + 2, rsl(rh)],
                                start=(kp == 0), stop=(kp == kpn - 1),
                                perf_mode=DR)
                    for rh in range(2):
                        pending.append((mc, rh, ps[rh]))
                        if len(pending) > defer:
                            evict(*pending.pop(0))
                while pending:
                    evict(*pending.pop(0))

            def sq_and_reduce(mc, rh, red):
                """Square the freshly written snew chunk; accumulate row
                sum-of-squares into the red psum via a ones-matmul."""
                sq = sqpool.tile([P, RH], BF, tag="sq")
                # on DVE (not ACT): keeps the ACT queue pure relu-evictions,
                # avoiding head-of-line blocking behind the DVE combine
                nc.vector.tensor_mul(sq[:], snew[:, mc, rsl(rh)],
                                     snew[:, mc, rsl(rh)])
                nc.tensor.matmul(red[rh][:], ones_red[:], sq[:],
                                 start=(mc == 0), stop=(mc == MC - 1))

            def finale(red, tgt, goodness):
                """red[rh] holds sum(snew^2) per row (scale S^2), already
                broadcast across all 128 partitions (M=128 ones-matmul).
                inv = G/(||snew|| + S*eps); tgt = snew*inv is the fp8
                normalized state at scale G."""
                if goodness:
                    for rh in range(2):
                        if goodness == "init":
                            nc.vector.tensor_copy(gacc[:, rsl(rh)],
                                                  red[rh][0:1, :])
                        else:
                            nc.vector.tensor_add(gacc[:, rsl(rh)],
                                                 gacc[:, rsl(rh)],
                                                 red[rh][0:1, :])
                if tgt is None:
                    return
                nr = small.tile([P, R], F32, tag="nr")
                for rh in range(2):
                    nc.scalar.activation(nr[:, rsl(rh)], red[rh][:],
                                         mybir.ActivationFunctionType.Sqrt,
                                         scale=1.0 / (G * G))
                nc.vector.tensor_scalar_add(nr[:], nr[:], S * EPS / G)
                inv = small.tile([P, R], F32, tag="inv")
                nc.vector.reciprocal_approx_fast(out=inv[:], in_=nr[:])
                for rh in range(2):
                    for mc in range(MC):
                        nc.vector.tensor_mul(tgt[:, mc, rsl(rh)],
                                             snew[:, mc, rsl(rh)],
                                             inv[:, rsl(rh)])

            def evict_to(dst, bidx):
                def ev(mc, rh, ps):
                    nc.scalar.activation(
                        dst[:, mc, rsl(rh)], ps[:],
                        mybir.ActivationFunctionType.Relu,
                        bias=bias_ap(bidx, mc))
                return ev

            def evict_add_comb(bidx):
                def ev(mc, rh, ps):
                    e = epool.tile([P, RH], F32, tag="e")
                    nc.scalar.activation(
                        e[:], ps[:], mybir.ActivationFunctionType.Relu,
                        bias=bias_ap(bidx, mc))
                    nc.vector.tensor_add(comb[:, mc, rsl(rh)],
                                         e[:], comb[:, mc, rsl(rh)])
                return ev

            # ---- A = relu(hxn @ w1pre' + S*0.7*b1pre), cached for all steps.
            # t0-n1 (snew = A + c1) is fused into the same pass so its
            # elementwise work overlaps the A matmuls chunk by chunk.
            red = red_pair()

            def ev_a(mc, rh, ps, red=red):
                nc.scalar.activation(
                    At[:, mc, rsl(rh)], ps[:],
                    mybir.ActivationFunctionType.Relu,
                    bias=bias_ap(B1PRE, mc))
                nc.vector.tensor_scalar_add(
                    snew[:, mc, rsl(rh)], At[:, mc, rsl(rh)],
                    bias_ap(C1, mc))
                sq_and_reduce(mc, rh, red)

            # defer=4: the A pass produces chunks quickly (4 k-pairs), so
            # the ~1.5us eviction chain needs extra slack to stay hidden
            term_pass("w1pre", KC1, hx, ev_a, w0_tile=w0, defer=4)
            finale(red, s1, None)

            # ---- t0, n2 / n3: single pre-term + const.
            # t1-n1's post/self term passes are wedged between them: they
            # only need s2(t0)/s1(t0) and don't touch comb (the t0 updates
            # don't use it), so their matmuls fill t0's serial-chain tails.
            def ev_t0(red, cidx, bpre):
                def ev(mc, rh, ps):
                    e = epool.tile([P, RH], F32, tag="e")
                    nc.scalar.activation(
                        e[:], ps[:], mybir.ActivationFunctionType.Relu,
                        bias=bias_ap(bpre, mc))
                    nc.vector.tensor_scalar_add(
                        snew[:, mc, rsl(rh)], e[:], bias_ap(cidx, mc))
                    sq_and_reduce(mc, rh, red)
                return ev

            red = red_pair()
            term_pass("w2pre", KC, s1, ev_t0(red, C2, B2PRE))
            finale(red, s2, None)

            term_pass("w1post", KC, s2, evict_to(comb, B1POST))
            term_pass("w1self", KC, s1, evict_add_comb(B1SELF))

            red = red_pair()
            term_pass("w3pre", KC, s2, ev_t0(red, C3, B3PRE))
            finale(red, s3, None)

            def n1_combine(last):
                red = red_pair()
                for mc in range(MC):
                    for rh in range(2):
                        nc.vector.tensor_add(snew[:, mc, rsl(rh)],
                                             At[:, mc, rsl(rh)],
                                             comb[:, mc, rsl(rh)])
                        sq_and_reduce(mc, rh, red)
                finale(red, s1, "init" if last else None)

            # ---- t1 / t2
            for t in (1, 2):
                last = (t == 2)
                # n1 = A + relu(s2@w1post'+b) + relu(s1@w1self'+b)
                if t == 2:
                    term_pass("w1post", KC, s2, evict_to(comb, B1POST))
                    term_pass("w1self", KC, s1, evict_add_comb(B1SELF))
                n1_combine(last)

                # n2 = relu(s1new@w2pre') + relu(s3@w2post') + relu(s2@w2self')
                term_pass("w2post", KC, s3, evict_to(comb, B2POST))
                term_pass("w2self", KC, s2, evict_add_comb(B2SELF))
                red = red_pair()

                def ev_n2(mc, rh, ps, red=red):
                    e = epool.tile([P, RH], F32, tag="e")
                    nc.scalar.activation(
                        e[:], ps[:], mybir.ActivationFunctionType.Relu,
                        bias=bias_ap(B2PRE, mc))
                    nc.vector.tensor_add(snew[:, mc, rsl(rh)],
                                         e[:], comb[:, mc, rsl(rh)])
                    sq_and_reduce(mc, rh, red)

                term_pass("w2pre", KC, s1, ev_n2)
                finale(red, s2, "add" if last else None)

                # n3 = relu(s2new@w3pre') + c3p + relu(s3@w3self')
                term_pass("w3self", KC, s3, evict_to(comb, B3SELF))
                red = red_pair()

                def ev_n3(mc, rh, ps, red=red):
                    e = epool.tile([P, RH], F32, tag="e")
                    nc.scalar.activation(
                        e[:], ps[:], mybir.ActivationFunctionType.Relu,
                        bias=bias_ap(B3PRE, mc))
                    nc.vector.scalar_tensor_tensor(
                        snew[:, mc, rsl(rh)], e[:], bias_ap(C3P, mc),
                        comb[:, mc, rsl(rh)],
                        op0=mybir.AluOpType.add, op1=mybir.AluOpType.add)
                    sq_and_reduce(mc, rh, red)

                term_pass("w3pre", KC, s2, ev_n3)
                finale(red, None if last else s3, "add" if last else None)

            # ---- goodness out: g = gacc / (2048 * S^2)
            gout = consts.tile([1, R], F32, tag="gout")
            nc.scalar.mul(gout[:], gacc[:], 1.0 / (H * S * S))
            nc.sync.dma_start(out=g_d[:], in_=gout[:])

    nc.compile()
    return nc


def _block_weight(w, scale, kcn):
    """[2048, d_in] float32 -> [MC, P, kcn, P] fp8 blocked for linear DMA:
    host_w[mc, p, kc, m] = scale * W[mc*128+m, kc*128+p]."""
    w = np.asarray(w, dtype=np.float32) * scale
    din = w.shape[1]
    if din < kcn * P:
        w = np.pad(w, ((0, 0), (0, kcn * P - din)))
    blk = w.reshape(MC, P, kcn, P).transpose(0, 3, 2, 1)
    return np.ascontiguousarray(np.clip(blk, -240.0, 240.0).astype(NPF8))


def _col(v):
    """[2048] -> [128, 16] (partition-major bias layout)."""
    return np.asarray(v, dtype=np.float32).reshape(MC, P).T


def prepare_inputs(inputs):
    """Host prep: overlay+normalize Hx, prescale/block weights, pack biases.
    Returns (shared_map, per_core_hx list)."""
    x = np.asarray(inputs["x"], dtype=np.float32)
    mx = x.max()
    base = x.copy()
    base[:, :NL] = 0.0
    hx = np.tile(base[None, :, :], (NL, 1, 1))
    for l in range(NL):
        hx[l, :, l] = mx
    hx = hx.reshape(ROWS, D_IN)
    n = np.linalg.norm(hx, axis=1, keepdims=True)
    hxn = (G / (n + EPS)) * hx
    hxn = np.pad(hxn, ((0, 0), (0, D_IN_PAD - D_IN)))

    per_core_hx = []
    for c in range(N_CORES):
        h = hxn[c * R:(c + 1) * R].T            # [1024, 640]
        h = h.reshape(KC1, P, R).transpose(1, 0, 2)
        per_core_hx.append(np.ascontiguousarray(
            np.clip(h, -240.0, 240.0).astype(NPF8)))

    wa = ALPHA
    shared = {
        "w1pre": _block_weight(inputs["w1_pre"], 0.7 * wa, KC1),
        "w1post": _block_weight(inputs["w1_post"], 0.7 * wa, KC),
        "w1self": _block_weight(inputs["w1_self"], 0.3 * wa, KC),
        "w2pre": _block_weight(inputs["w2_pre"], 0.7 * wa, KC),
        "w2post": _block_weight(inputs["w2_post"], 0.7 * wa, KC),
        "w2self": _block_weight(inputs["w2_self"], 0.3 * wa, KC),
        "w3pre": _block_weight(inputs["w3_pre"], 0.7 * wa, KC),
        "w3self": _block_weight(inputs["w3_self"], 0.3 * wa, KC),
    }

    relu = lambda a: np.maximum(np.asarray(a, dtype=np.float32), 0.0)

    cols = np.empty((P, NBIAS * MC), dtype=np.float32)
    vals = {
        B1PRE: S * 0.7 * np.asarray(inputs["b1_pre"], np.float32),
        B1POST: S * 0.7 * np.asarray(inputs["b1_post"], np.float32),
        B1SELF: S * 0.3 * np.asarray(inputs["b1_self"], np.float32),
        B2PRE: S * 0.7 * np.asarray(inputs["b2_pre"], np.float32),
        B2POST: S * 0.7 * np.asarray(inputs["b2_post"], np.float32),
        B2SELF: S * 0.3 * np.asarray(inputs["b2_self"], np.float32),
        B3PRE: S * 0.7 * np.asarray(inputs["b3_pre"], np.float32),
        B3SELF: S * 0.3 * np.asarray(inputs["b3_self"], np.float32),
        C1: S * (0.7 * relu(inputs["b1_post"]) + 0.3 * relu(inputs["b1_self"])),
        C2: S * (0.7 * relu(inputs["b2_post"]) + 0.3 * relu(inputs["b2_self"])),
        C3: S * (0.7 * relu(inputs["b3_post"]) + 0.3 * relu(inputs["b3_self"])),
        C3P: S * 0.7 * relu(inputs["b3_post"]),
    }
    for idx, v in vals.items():
        cols[:, idx * MC:(idx + 1) * MC] = _col(v)
    shared["biases"] = np.ascontiguousarray(cols)

    return shared, per_core_hx


def run(inputs, trace=False):
    shared, per_core_hx = prepare_inputs(inputs)
    if "nc" not in _NC_CACHE:
        _NC_CACHE["nc"] = _build_nc()
    nc = _NC_CACHE["nc"]
    in_maps = [dict(shared, hxn=per_core_hx[c]) for c in range(N_CORES)]
    res = run_bass_kernel_spmd(nc, in_maps, core_ids=list(range(N_CORES)),
                               trace=trace)
    g = np.concatenate([res.results[c]["g"][0] for c in range(N_CORES)])
    out = g.reshape(NL, B).T.astype(np.float32)
    return np.ascontiguousarray(out), res


def kernel(**inputs):
    out, _ = run(inputs, trace=False)
    return out


# revision 4
# speedup vs baseline: 1.8346x; 1.0045x over previous
"""Trainium2 Bass kernel for the 3-metalayer forward-forward style MLP.

Distribution: the (10 labels x 512 batch) grid flattens to 5120 independent
rows; each of the 8 cores processes 640 rows (pure data parallelism, weights
replicated, no collectives).

Device-side algorithm (per core, rows R=640):
  - matmul inputs (states, overlay input, weights) are fp8 e4m3; every
    linear term runs as DoubleRow matmuls (K=256 per instruction, 2 fp8
    weights per PE cell -> 2x MACs/cycle).
  - states are stored UNNORMALIZED at gain G=64; the row normalization is
    applied to the matmul OUTPUT instead of the input (inv is per-row, so
    it commutes through the matmul): a DVE psum*inv multiply feeds the ACT
    relu+bias eviction. This removes the normalize->matmul serial
    dependency between passes - the next pass streams as soon as state
    chunks are written, and inv lands during its first psum groups.
  - weights carry gain ALPHA=1024 and the 0.7/0.3 metalayer blend (relu
    positive homogeneity); inv = (G/ALPHA)/(sqrt(ss)+G*eps) folds all
    gains; goodness = ss/(2048*G^2).
  - row L2 norms: square (DVE, fp8 out) + fp8 DoubleRow ones-matmul
    reduction over partition pairs (8 matmuls per 2048 features, M=128
    broadcasts the sum to every partition for free).
  - t=0 terms with zero-state inputs are host-folded constants; the layer-1
    "pre" term (static overlay input, host-prenormalized) is computed once
    and reused all 3 steps.
"""

import numpy as np
import ml_dtypes

import concourse.bass as bass
import concourse.tile as tile
from concourse import bacc, mybir
from concourse.bass_utils import run_bass_kernel_spmd

BF = mybir.dt.bfloat16
F32 = mybir.dt.float32
FP8 = mybir.dt.float8e4
NPBF = ml_dtypes.bfloat16
NPF8 = ml_dtypes.float8_e4m3
DR = mybir.MatmulPerfMode.DoubleRow

N_CORES = 8
P = 128
D_IN = 784
D_IN_PAD = 1024           # 8 * 128 (even k-chunk count for DoubleRow pairs)
KC1 = 8                   # k-chunks for the 784->2048 matmul (padded)
KC = 16                   # k-chunks for 2048-contraction matmuls
MC = 16                   # output-feature chunks (2048 / 128)
H = 2048
B = 512
NL = 10
ROWS = NL * B             # 5120
R = ROWS // N_CORES       # 640 rows per core
RH = 320                  # psum row-chunk (2 per core-row-block)
EPS = 1e-4

ALPHA = 1024.0            # weight fp8 gain
G = 64.0                  # state fp8 gain

# bias/const column indices inside the packed [128, 12*16] bias tensor
B1PRE, B1POST, B1SELF, B2PRE, B2POST, B2SELF, B3PRE, B3SELF, C1, C2, C3, C3P = range(12)
NBIAS = 12

_NC_CACHE = {}


def _build_nc():
    """Build the single-core Tile program (same NEFF for all 8 cores)."""
    nc = bacc.Bacc("TRN2", target_bir_lowering=False, debug=False,
                   num_devices=N_CORES)

    hx_d = nc.dram_tensor("hxn", [P, KC1, R], FP8, kind="ExternalInput")
    w_d = {
        "w1pre": nc.dram_tensor("w1pre", [MC, P, KC1, P], FP8, kind="ExternalInput"),
    }
    for name in ("w1post", "w1self", "w2pre", "w2post", "w2self", "w3pre", "w3self"):
        w_d[name] = nc.dram_tensor(name, [MC, P, KC, P], FP8, kind="ExternalInput")
    bias_d = nc.dram_tensor("biases", [P, NBIAS * MC], F32, kind="ExternalInput")
    g_d = nc.dram_tensor("g", [1, R], F32, kind="ExternalOutput")

    with tile.TileContext(nc) as tc:
        with (
            tc.tile_pool(name="consts", bufs=1) as consts,
            tc.tile_pool(name="states", bufs=1) as states,
            tc.tile_pool(name="wpool", bufs=8) as wpool,
            tc.tile_pool(name="epool", bufs=8) as epool,
            tc.tile_pool(name="sqpool", bufs=4) as sqpool,
            tc.tile_pool(name="small", bufs=2) as small,
            tc.tile_pool(name="mmps", bufs=6, space="PSUM") as mmps,
            tc.tile_pool(name="redps", bufs=2, space="PSUM") as redps,
        ):
            # startup order: first hx chunk + first weight block must land
            # before anything else so the PE starts within ~1.5us
            hx = states.tile([P, KC1, R], FP8, tag="hxn")
            nc.sync.dma_start(out=hx[:, 0:2, :], in_=hx_d[:, 0:2, :])
            bias_sb = consts.tile([P, NBIAS * MC], F32)
            w0 = wpool.tile([P, KC1, P], FP8, tag="w", name="w1pre0")
            nc.sync.dma_start(out=w0[:], in_=w_d["w1pre"][0])
            nc.sync.dma_start(out=bias_sb[:], in_=bias_d[:])
            for kc in range(2, KC1, 2):
                nc.sync.dma_start(out=hx[:, kc:kc + 2, :], in_=hx_d[:, kc:kc + 2, :])
            # [128, 2, 128] fp8 ones: M=128 DoubleRow ones-matmul reduces a
            # partition-pair AND broadcasts the row sum to every partition
            ones8 = consts.tile([P, 2, P], FP8)
            nc.vector.memset(ones8[:], 1.0)
            gacc = consts.tile([1, R], F32)

            # warm the PE HAM clock gate while the initial DMAs are in
            # flight: the dummy matmuls span >3.4us of PE activity, so the
            # real matmul stream starts at 2.4GHz instead of 1.2GHz
            warm_ps = mmps.tile([P, RH], F32, tag="mm", name="warm_ps")
            for _ in range(64):
                nc.tensor.matmul(warm_ps[:, :P], ones8[:, 0, :], ones8[:, 0, :],
                                 start=True, stop=True)
            At = states.tile([P, MC, R], FP8, tag="A")
            s1 = states.tile([P, MC, R], FP8, tag="s1")
            s2 = states.tile([P, MC, R], FP8, tag="s2")
            s3 = states.tile([P, MC, R], FP8, tag="s3")
            comb = states.tile([P, MC, R], BF, tag="comb")
            inv1 = states.tile([P, R], F32, tag="inv1")
            inv2 = states.tile([P, R], F32, tag="inv2")
            inv3 = states.tile([P, R], F32, tag="inv3")

            _red_uid = [0]

            def red_pair():
                _red_uid[0] += 1
                u = _red_uid[0]
                return (redps.tile([P, RH], F32, tag="red", name=f"red{u}a"),
                        redps.tile([P, RH], F32, tag="red", name=f"red{u}b"))

            def bias_ap(idx, mc):
                col = idx * MC + mc
                return bias_sb[:, col:col + 1]

            def rsl(rh):
                return slice(rh * RH, (rh + 1) * RH)

            def term_pass(wname, kcn, src, evict, w0_tile=None, defer=2):
                """One linear term: stream weight blocks, accumulate psums,
                hand each [128, RH] psum chunk to `evict(mc, rh, ps)`.

                DoubleRow: each matmul consumes a k-pair (K=256); the two
                row-halves run back-to-back on the same stationary weights.

                Evictions are emitted `defer` psum-groups late: the eviction
                chain (DVE inv-mul -> ACT relu -> DVE combine) has ~1.5us of
                cross-engine latency, and emitting it inline makes the
                strict-FIFO PE queue stall. Deferring places it behind
                independent matmul work."""
                wd = w_d[wname]
                kpn = kcn // 2
                pending = []
                for mc in range(MC):
                    if mc == 0 and w0_tile is not None:
                        wt = w0_tile
                    else:
                        wt = wpool.tile([P, kcn, P], FP8, tag="w")
                        nc.sync.dma_start(out=wt[:], in_=wd[mc])
                    ps = [mmps.tile([P, RH], F32, tag="mm",
                                    name=f"mm_{wname}_{mc}_{rh}")
                          for rh in range(2)]
                    for kp in range(kpn):
                        for rh in range(2):
                            nc.tensor.matmul(
                                ps[rh][:], wt[:, 2 * kp:2 * kp + 2, :],
                                src[:, 2 * kp:2 * kp + 2, rsl(rh)],
                                start=(kp == 0), stop=(kp == kpn - 1),
                                perf_mode=DR)
                    for rh in range(2):
                        pending.append((mc, rh, ps[rh]))
                        if len(pending) > defer:
                            evict(*pending.pop(0))
                while pending:
                    evict(*pending.pop(0))

            _sqt = [None]

            def sq_and_reduce(mc, st, red):
                """After both row-halves of state chunk mc are written:
                square it (fp8), and on odd mc reduce the pair into the red
                psum with a DoubleRow ones-matmul."""
                if mc % 2 == 0:
                    _sqt[0] = sqpool.tile([P, 2, R], FP8, tag="sq",
                                          name=f"sq{mc}")
                sqt = _sqt[0]
                nc.vector.tensor_mul(sqt[:, mc % 2, :], st[:, mc, :],
                                     st[:, mc, :])
                if mc % 2 == 1:
                    pair = mc // 2
                    for rh in range(2):
                        nc.tensor.matmul(red[rh][:], ones8[:],
                                         sqt[:, :, rsl(rh)],
                                         start=(pair == 0), stop=(pair == 7),
                                         perf_mode=DR)

            def finale(red, inv_t, goodness):
                """red[rh] holds ss = sum(s8^2) per row (gain G^2), already
                broadcast across all 128 partitions. inv_t (if wanted) gets
                (G/ALPHA)/(sqrt(ss)+G*eps), the per-row factor that turns
                a raw psum into the normalized-input result."""
                if goodness:
                    for rh in range(2):
                        if goodness == "init":
                            nc.vector.tensor_copy(gacc[:, rsl(rh)],
                                                  red[rh][0:1, :])
                        else:
                            nc.vector.tensor_add(gacc[:, rsl(rh)],
                                                 gacc[:, rsl(rh)],
                                                 red[rh][0:1, :])
                if inv_t is None:
                    return
                nr = small.tile([P, R], F32, tag="nr")
                for rh in range(2):
                    nc.scalar.activation(nr[:, rsl(rh)], red[rh][:],
                                         mybir.ActivationFunctionType.Sqrt,
                                         scale=(ALPHA / G) ** 2)
                nc.vector.tensor_scalar_add(nr[:], nr[:], ALPHA * EPS)
                nc.vector.reciprocal_approx_fast(out=inv_t[:], in_=nr[:])

            def inv_mul(ps, inv_t, rh):
                e2 = epool.tile([P, RH], F32, tag="e2")
                nc.vector.tensor_mul(e2[:], ps[:], inv_t[:, rsl(rh)])
                return e2

            def evict_to(dst, bidx, inv_t):
                def ev(mc, rh, ps):
                    e2 = inv_mul(ps, inv_t, rh)
                    nc.scalar.activation(
                        dst[:, mc, rsl(rh)], e2[:],
                        mybir.ActivationFunctionType.Relu,
                        bias=bias_ap(bidx, mc))
                return ev

            def evict_add_comb(bidx, inv_t):
                def ev(mc, rh, ps):
                    e2 = inv_mul(ps, inv_t, rh)
                    e = epool.tile([P, RH], F32, tag="e")
                    nc.scalar.activation(
                        e[:], e2[:], mybir.ActivationFunctionType.Relu,
                        bias=bias_ap(bidx, mc))
                    nc.vector.tensor_add(comb[:, mc, rsl(rh)],
                                         e[:], comb[:, mc, rsl(rh)])
                return ev

            # ---- A = relu((hxn @ w1pre')/ALPHA + G*0.7*b1pre), cached for
            # all steps (hx is host-prenormalized: no inv needed).
            # t0-n1 (s1 = A + c1) is fused into the same pass.
            red = red_pair()

            def ev_a(mc, rh, ps, red=red):
                nc.scalar.activation(
                    At[:, mc, rsl(rh)], ps[:],
                    mybir.ActivationFunctionType.Relu,
                    bias=bias_ap(B1PRE, mc), scale=1.0 / ALPHA)
                nc.vector.tensor_scalar_add(
                    s1[:, mc, rsl(rh)], At[:, mc, rsl(rh)],
                    bias_ap(C1, mc))
                if rh == 1:
                    sq_and_reduce(mc, s1, red)

            # defer=4: the A pass produces chunks quickly (4 k-pairs), so
            # the ~1.5us eviction chain needs extra slack to stay hidden
            term_pass("w1pre", KC1, hx, ev_a, w0_tile=w0, defer=4)
            finale(red, inv1, None)

            # ---- t0, n2 / n3: single pre-term + const.
            # t1-n1's post/self term passes are wedged between them: they
            # only need s2(t0)/s1(t0) and don't touch comb (the t0 updates
            # don't use it), so their matmuls fill t0's serial-chain tails.
            def ev_t0(red, tgt, inv_t, cidx, bpre):
                def ev(mc, rh, ps):
                    e2 = inv_mul(ps, inv_t, rh)
                    e = epool.tile([P, RH], F32, tag="e")
                    nc.scalar.activation(
                        e[:], e2[:], mybir.ActivationFunctionType.Relu,
                        bias=bias_ap(bpre, mc))
                    nc.vector.tensor_scalar_add(
                        tgt[:, mc, rsl(rh)], e[:], bias_ap(cidx, mc))
                    if rh == 1:
                        sq_and_reduce(mc, tgt, red)
                return ev

            red = red_pair()
            term_pass("w2pre", KC, s1, ev_t0(red, s2, inv1, C2, B2PRE))
            finale(red, inv2, None)

            term_pass("w1post", KC, s2, evict_to(comb, B1POST, inv2))
            term_pass("w1self", KC, s1, evict_add_comb(B1SELF, inv1))

            red = red_pair()
            term_pass("w3pre", KC, s2, ev_t0(red, s3, inv2, C3, B3PRE))
            finale(red, inv3, None)

            def n1_combine(last):
                red = red_pair()
                for mc in range(MC):
                    nc.vector.tensor_add(s1[:, mc, :], At[:, mc, :],
                                         comb[:, mc, :])
                    sq_and_reduce(mc, s1, red)
                finale(red, inv1, "init" if last else None)

            # ---- t1 / t2
            for t in (1, 2):
                last = (t == 2)
                # n1 = A + relu(s2@w1post'+b) + relu(s1@w1self'+b)
                if t == 2:
                    term_pass("w1post", KC, s2, evict_to(comb, B1POST, inv2))
                    term_pass("w1self", KC, s1, evict_add_comb(B1SELF, inv1))
                n1_combine(last)

                # n2 = relu(s1new@w2pre') + relu(s3@w2post') + relu(s2@w2self')
                term_pass("w2post", KC, s3, evict_to(comb, B2POST, inv3))
                term_pass("w2self", KC, s2, evict_add_comb(B2SELF, inv2))
                red = red_pair()

                def ev_n2(mc, rh, ps, red=red):
                    e2 = inv_mul(ps, inv1, rh)
                    e = epool.tile([P, RH], F32, tag="e")
                    nc.scalar.activation(
                        e[:], e2[:], mybir.ActivationFunctionType.Relu,
                        bias=bias_ap(B2PRE, mc))
                    nc.vector.tensor_add(s2[:, mc, rsl(rh)],
                                         e[:], comb[:, mc, rsl(rh)])
                    if rh == 1:
                        sq_and_reduce(mc, s2, red)

                term_pass("w2pre", KC, s1, ev_n2)
                finale(red, inv2, "add" if last else None)

                # n3 = relu(s2new@w3pre') + c3p + relu(s3@w3self')
                term_pass("w3self", KC, s3, evict_to(comb, B3SELF, inv3))
                red = red_pair()

                def ev_n3(mc, rh, ps, red=red):
                    e2 = inv_mul(ps, inv2, rh)
                    e = epool.tile([P, RH], F32, tag="e")
                    nc.scalar.activation(
                        e[:], e2[:], mybir.ActivationFunctionType.Relu,
                        bias=bias_ap(B3PRE, mc))
                    nc.vector.scalar_tensor_tensor(
                        s3[:, mc, rsl(rh)], e[:], bias_ap(C3P, mc),
                        comb[:, mc, rsl(rh)],
                        op0=mybir.AluOpType.add, op1=mybir.AluOpType.add)
                    if rh == 1:
                        sq_and_reduce(mc, s3, red)

                term_pass("w3pre", KC, s2, ev_n3)
                finale(red, None if last else inv3, "add" if last else None)

            # ---- goodness out: g = gacc / (2048 * G^2)
            gout = consts.tile([1, R], F32, tag="gout")
            nc.scalar.mul(gout[:], gacc[:], 1.0 / (H * G * G))
            nc.sync.dma_start(out=g_d[:], in_=gout[:])

    nc.compile()
    return nc


def _block_weight(w, scale, kcn):
    """[2048, d_in] float32 -> [MC, P, kcn, P] fp8 blocked for linear DMA:
    host_w[mc, p, kc, m] = scale * W[mc*128+m, kc*128+p]."""
    w = np.asarray(w, dtype=np.float32) * scale
    din = w.shape[1]
    if din < kcn * P:
        w = np.pad(w, ((0, 0), (0, kcn * P - din)))
    blk = w.reshape(MC, P, kcn, P).transpose(0, 3, 2, 1)
    return np.ascontiguousarray(np.clip(blk, -240.0, 240.0).astype(NPF8))


def _col(v):
    """[2048] -> [128, 16] (partition-major bias layout)."""
    return np.asarray(v, dtype=np.float32).reshape(MC, P).T


def prepare_inputs(inputs):
    """Host prep: overlay+normalize Hx, prescale/block weights, pack biases.
    Returns (shared_map, per_core_hx list)."""
    x = np.asarray(inputs["x"], dtype=np.float32)
    mx = x.max()
    base = x.copy()
    base[:, :NL] = 0.0
    hx = np.tile(base[None, :, :], (NL, 1, 1))
    for l in range(NL):
        hx[l, :, l] = mx
    hx = hx.reshape(ROWS, D_IN)
    n = np.linalg.norm(hx, axis=1, keepdims=True)
    hxn = (G / (n + EPS)) * hx
    hxn = np.pad(hxn, ((0, 0), (0, D_IN_PAD - D_IN)))

    per_core_hx = []
    for c in range(N_CORES):
        h = hxn[c * R:(c + 1) * R].T            # [1024, 640]
        h = h.reshape(KC1, P, R).transpose(1, 0, 2)
        per_core_hx.append(np.ascontiguousarray(
            np.clip(h, -240.0, 240.0).astype(NPF8)))

    wa = ALPHA
    shared = {
        "w1pre": _block_weight(inputs["w1_pre"], 0.7 * wa, KC1),
        "w1post": _block_weight(inputs["w1_post"], 0.7 * wa, KC),
        "w1self": _block_weight(inputs["w1_self"], 0.3 * wa, KC),
        "w2pre": _block_weight(inputs["w2_pre"], 0.7 * wa, KC),
        "w2post": _block_weight(inputs["w2_post"], 0.7 * wa, KC),
        "w2self": _block_weight(inputs["w2_self"], 0.3 * wa, KC),
        "w3pre": _block_weight(inputs["w3_pre"], 0.7 * wa, KC),
        "w3self": _block_weight(inputs["w3_self"], 0.3 * wa, KC),
    }

    relu = lambda a: np.maximum(np.asarray(a, dtype=np.float32), 0.0)

    cols = np.empty((P, NBIAS * MC), dtype=np.float32)
    vals = {
        B1PRE: G * 0.7 * np.asarray(inputs["b1_pre"], np.float32),
        B1POST: G * 0.7 * np.asarray(inputs["b1_post"], np.float32),
        B1SELF: G * 0.3 * np.asarray(inputs["b1_self"], np.float32),
        B2PRE: G * 0.7 * np.asarray(inputs["b2_pre"], np.float32),
        B2POST: G * 0.7 * np.asarray(inputs["b2_post"], np.float32),
        B2SELF: G * 0.3 * np.asarray(inputs["b2_self"], np.float32),
        B3PRE: G * 0.7 * np.asarray(inputs["b3_pre"], np.float32),
        B3SELF: G * 0.3 * np.asarray(inputs["b3_self"], np.float32),
        C1: G * (0.7 * relu(inputs["b1_post"]) + 0.3 * relu(inputs["b1_self"])),
        C2: G * (0.7 * relu(inputs["b2_post"]) + 0.3 * relu(inputs["b2_self"])),
        C3: G * (0.7 * relu(inputs["b3_post"]) + 0.3 * relu(inputs["b3_self"])),
        C3P: G * 0.7 * relu(inputs["b3_post"]),
    }
    for idx, v in vals.items():
        cols[:, idx * MC:(idx + 1) * MC] = _col(v)
    shared["biases"] = np.ascontiguousarray(cols)

    return shared, per_core_hx


def run(inputs, trace=False):
    shared, per_core_hx = prepare_inputs(inputs)
    if "nc" not in _NC_CACHE:
        _NC_CACHE["nc"] = _build_nc()
    nc = _NC_CACHE["nc"]
    in_maps = [dict(shared, hxn=per_core_hx[c]) for c in range(N_CORES)]
    res = run_bass_kernel_spmd(nc, in_maps, core_ids=list(range(N_CORES)),
                               trace=trace)
    g = np.concatenate([res.results[c]["g"][0] for c in range(N_CORES)])
    out = g.reshape(NL, B).T.astype(np.float32)
    return np.ascontiguousarray(out), res


def kernel(**inputs):
    out, _ = run(inputs, trace=False)
    return out


# revision 5
# speedup vs baseline: 1.9411x; 1.0580x over previous
"""Trainium2 Bass kernel for the 3-metalayer forward-forward style MLP.

Distribution: the (10 labels x 512 batch) grid flattens to 5120 independent
rows; each of the 8 cores processes 640 rows (pure data parallelism, weights
replicated, no collectives).

Device-side algorithm (per core, rows R=640):
  - matmul inputs (states, overlay input, weights) are fp8 e4m3; every
    linear term runs as DoubleRow matmuls (K=256 per instruction, 2 fp8
    weights per PE cell -> 2x MACs/cycle).
  - the working state snew is bf16 at gain G=64; its fp8 copy s_i (made by
    an ACT dtype-convert) feeds the matmuls. The row normalization is
    applied to the matmul OUTPUT (inv is per-row, so it commutes through
    the matmul): a DVE psum*inv multiply feeds the ACT relu+bias eviction.
    This removes the normalize->matmul serial dependency between passes.
    Norms/goodness come from the bf16 snew, so fp8 quantization noise does
    not bias the normalization (no coherent rho^2/2 shrink).
  - weights carry gain ALPHA=1024 and the 0.7/0.3 metalayer blend (relu
    positive homogeneity); inv = (G/ALPHA)/(sqrt(ss)+G*eps) folds all
    gains; goodness = ss/(2048*G^2).
  - row L2 norms: square (DVE, fp8 out) + fp8 DoubleRow ones-matmul
    reduction over partition pairs (8 matmuls per 2048 features, M=128
    broadcasts the sum to every partition for free).
  - t=0 terms with zero-state inputs are host-folded constants; the layer-1
    "pre" term (static overlay input, host-prenormalized) is computed once
    and reused all 3 steps; the t1/t2 n1 combine (pure DVE work) is wedged
    mc-by-mc into the following w2post pass so the PE never idles on it.
"""

import numpy as np
import ml_dtypes

import concourse.bass as bass
import concourse.tile as tile
from concourse import bacc, mybir
from concourse.bass_utils import run_bass_kernel_spmd

BF = mybir.dt.bfloat16
F32 = mybir.dt.float32
FP8 = mybir.dt.float8e4
NPBF = ml_dtypes.bfloat16
NPF8 = ml_dtypes.float8_e4m3
DR = mybir.MatmulPerfMode.DoubleRow

N_CORES = 8
P = 128
D_IN = 784
D_IN_PAD = 1024           # 8 * 128 (even k-chunk count for DoubleRow pairs)
KC1 = 8                   # k-chunks for the 784->2048 matmul (padded)
KC = 16                   # k-chunks for 2048-contraction matmuls
MC = 16                   # output-feature chunks (2048 / 128)
H = 2048
B = 512
NL = 10
ROWS = NL * B             # 5120
R = ROWS // N_CORES       # 640 rows per core
RH = 320                  # psum row-chunk (2 per core-row-block)
EPS = 1e-4

ALPHA = 1024.0            # weight fp8 gain
G = 64.0                  # state gain

# bias/const column indices inside the packed [128, 12*16] bias tensor
B1PRE, B1POST, B1SELF, B2PRE, B2POST, B2SELF, B3PRE, B3SELF, C1, C2, C3, C3P = range(12)
NBIAS = 12

_NC_CACHE = {}


def _build_nc():
    """Build the single-core Tile program (same NEFF for all 8 cores)."""
    nc = bacc.Bacc("TRN2", target_bir_lowering=False, debug=False,
                   num_devices=N_CORES)

    hx_d = nc.dram_tensor("hxn", [P, KC1, R], FP8, kind="ExternalInput")
    w_d = {
        "w1pre": nc.dram_tensor("w1pre", [MC, P, KC1, P], FP8, kind="ExternalInput"),
    }
    for name in ("w1post", "w1self", "w2pre", "w2post", "w2self", "w3pre", "w3self"):
        w_d[name] = nc.dram_tensor(name, [MC, P, KC, P], FP8, kind="ExternalInput")
    bias_d = nc.dram_tensor("biases", [P, NBIAS * MC], F32, kind="ExternalInput")
    g_d = nc.dram_tensor("g", [1, R], F32, kind="ExternalOutput")

    with tile.TileContext(nc) as tc:
        with (
            tc.tile_pool(name="consts", bufs=1) as consts,
            tc.tile_pool(name="states", bufs=1) as states,
            tc.tile_pool(name="wpool", bufs=12) as wpool,
            tc.tile_pool(name="epool", bufs=8) as epool,
            tc.tile_pool(name="sqpool", bufs=4) as sqpool,
            tc.tile_pool(name="small", bufs=2) as small,
            tc.tile_pool(name="mmps", bufs=6, space="PSUM") as mmps,
            tc.tile_pool(name="redps", bufs=2, space="PSUM") as redps,
        ):
            # startup order: first hx chunk + first weight block must land
            # before anything else so the PE starts within ~1.5us
            hx = states.tile([P, KC1, R], FP8, tag="hxn")
            nc.sync.dma_start(out=hx[:, 0:2, :], in_=hx_d[:, 0:2, :])
            bias_sb = consts.tile([P, NBIAS * MC], F32)
            w0 = wpool.tile([P, KC1, P], FP8, tag="w", name="w1pre0")
            nc.sync.dma_start(out=w0[:], in_=w_d["w1pre"][0])
            nc.sync.dma_start(out=bias_sb[:], in_=bias_d[:])
            for kc in range(2, KC1, 2):
                nc.sync.dma_start(out=hx[:, kc:kc + 2, :], in_=hx_d[:, kc:kc + 2, :])
            # [128, 2, 128] fp8 ones: M=128 DoubleRow ones-matmul reduces a
            # partition-pair AND broadcasts the row sum to every partition
            ones8 = consts.tile([P, 2, P], FP8)
            nc.vector.memset(ones8[:], 1.0)
            gacc = consts.tile([1, R], F32)

            # warm the PE HAM clock gate while the initial DMAs are in
            # flight: the dummy matmuls span >3.4us of PE activity, so the
            # real matmul stream starts at 2.4GHz instead of 1.2GHz
            warm_ps = mmps.tile([P, RH], F32, tag="mm", name="warm_ps")
            for _ in range(64):
                nc.tensor.matmul(warm_ps[:, :P], ones8[:, 0, :], ones8[:, 0, :],
                                 start=True, stop=True)
            At = states.tile([P, MC, R], BF, tag="A")
            snew = states.tile([P, MC, R], BF, tag="snew")
            s1 = states.tile([P, MC, R], FP8, tag="s1")
            s2 = states.tile([P, MC, R], FP8, tag="s2")
            s3 = states.tile([P, MC, R], FP8, tag="s3")
            comb = states.tile([P, MC, R], BF, tag="comb")
            inv1 = states.tile([P, R], F32, tag="inv1")
            inv2 = states.tile([P, R], F32, tag="inv2")
            inv3 = states.tile([P, R], F32, tag="inv3")

            _red_uid = [0]

            def red_pair():
                _red_uid[0] += 1
                u = _red_uid[0]
                return (redps.tile([P, RH], F32, tag="red", name=f"red{u}a"),
                        redps.tile([P, RH], F32, tag="red", name=f"red{u}b"))

            def bias_ap(idx, mc):
                col = idx * MC + mc
                return bias_sb[:, col:col + 1]

            def rsl(rh):
                return slice(rh * RH, (rh + 1) * RH)

            def term_pass(wname, kcn, src, evict, w0_tile=None, defer=2,
                          wedge=None):
                """One linear term: stream weight blocks, accumulate psums,
                hand each [128, RH] psum chunk to `evict(mc, rh, ps)`.

                DoubleRow: each matmul consumes a k-pair (K=256); the two
                row-halves run back-to-back on the same stationary weights.

                Evictions are emitted `defer` psum-groups late: the eviction
                chain (DVE inv-mul -> ACT relu -> DVE combine) has ~1.5us of
                cross-engine latency, and emitting it inline makes the
                strict-FIFO PE queue stall. Deferring places it behind
                independent matmul work.

                `wedge(mc)`, if given, emits independent non-PE work after
                each mc block's matmuls (used to hide the n1 combine)."""
                wd = w_d[wname]
                kpn = kcn // 2
                pending = []
                for mc in range(MC):
                    if mc == 0 and w0_tile is not None:
                        wt = w0_tile
                    else:
                        wt = wpool.tile([P, kcn, P], FP8, tag="w")
                        nc.sync.dma_start(out=wt[:], in_=wd[mc])
                    ps = [mmps.tile([P, RH], F32, tag="mm",
                                    name=f"mm_{wname}_{mc}_{rh}")
                          for rh in range(2)]
                    for kp in range(kpn):
                        for rh in range(2):
                            nc.tensor.matmul(
                                ps[rh][:], wt[:, 2 * kp:2 * kp + 2, :],
                                src[:, 2 * kp:2 * kp + 2, rsl(rh)],
                                start=(kp == 0), stop=(kp == kpn - 1),
                                perf_mode=DR)
                    if wedge is not None:
                        wedge(mc)
                    for rh in range(2):
                        pending.append((mc, rh, ps[rh]))
                        if len(pending) > defer:
                            evict(*pending.pop(0))
                while pending:
                    evict(*pending.pop(0))

            _sqt = [None]

            def store_sq_reduce(mc, s8, red):
                """After both row-halves of snew chunk mc are written: make
                the fp8 matmul copy, square (fp8), and on odd mc reduce the
                pair into the red psum with a DoubleRow ones-matmul."""
                nc.scalar.copy(s8[:, mc, :], snew[:, mc, :])
                if mc % 2 == 0:
                    _sqt[0] = sqpool.tile([P, 2, R], FP8, tag="sq",
                                          name=f"sq{mc}")
                sqt = _sqt[0]
                nc.vector.tensor_mul(sqt[:, mc % 2, :], snew[:, mc, :],
                                     snew[:, mc, :])
                if mc % 2 == 1:
                    pair = mc // 2
                    for rh in range(2):
                        nc.tensor.matmul(red[rh][:], ones8[:],
                                         sqt[:, :, rsl(rh)],
                                         start=(pair == 0), stop=(pair == 7),
                                         perf_mode=DR)

            def finale(red, inv_t, goodness):
                """red[rh] holds ss = sum(snew^2) per row (gain G^2), already
                broadcast across all 128 partitions. inv_t (if wanted) gets
                (G/ALPHA)/(sqrt(ss)+G*eps), the per-row factor that turns
                a raw psum into the normalized-input result."""
                if goodness:
                    for rh in range(2):
                        if goodness == "init":
                            nc.vector.tensor_copy(gacc[:, rsl(rh)],
                                                  red[rh][0:1, :])
                        else:
                            nc.vector.tensor_add(gacc[:, rsl(rh)],
                                                 gacc[:, rsl(rh)],
                                                 red[rh][0:1, :])
                if inv_t is None:
                    return
                nr = small.tile([P, R], F32, tag="nr")
                for rh in range(2):
                    nc.scalar.activation(nr[:, rsl(rh)], red[rh][:],
                                         mybir.ActivationFunctionType.Sqrt,
                                         scale=(ALPHA / G) ** 2)
                nc.vector.tensor_scalar_add(nr[:], nr[:], ALPHA * EPS)
                nc.vector.reciprocal_approx_fast(out=inv_t[:], in_=nr[:])

            def inv_mul(ps, inv_t, rh):
                e2 = epool.tile([P, RH], F32, tag="e2")
                nc.vector.tensor_mul(e2[:], ps[:], inv_t[:, rsl(rh)])
                return e2

            def evict_to(dst, bidx, inv_t):
                def ev(mc, rh, ps):
                    e2 = inv_mul(ps, inv_t, rh)
                    nc.scalar.activation(
                        dst[:, mc, rsl(rh)], e2[:],
                        mybir.ActivationFunctionType.Relu,
                        bias=bias_ap(bidx, mc))
                return ev

            def evict_add_comb(bidx, inv_t):
                def ev(mc, rh, ps):
                    e2 = inv_mul(ps, inv_t, rh)
                    e = epool.tile([P, RH], F32, tag="e")
                    nc.scalar.activation(
                        e[:], e2[:], mybir.ActivationFunctionType.Relu,
                        bias=bias_ap(bidx, mc))
                    nc.vector.tensor_add(comb[:, mc, rsl(rh)],
                                         e[:], comb[:, mc, rsl(rh)])
                return ev

            # ---- A = relu((hxn @ w1pre')/ALPHA + G*0.7*b1pre), cached for
            # all steps (hx is host-prenormalized: no inv needed).
            # t0-n1 (snew = A + c1) is fused into the same pass.
            red = red_pair()

            def ev_a(mc, rh, ps, red=red):
                nc.scalar.activation(
                    At[:, mc, rsl(rh)], ps[:],
                    mybir.ActivationFunctionType.Relu,
                    bias=bias_ap(B1PRE, mc), scale=1.0 / ALPHA)
                nc.vector.tensor_scalar_add(
                    snew[:, mc, rsl(rh)], At[:, mc, rsl(rh)],
                    bias_ap(C1, mc))
                if rh == 1:
                    store_sq_reduce(mc, s1, red)

            # defer=4: the A pass produces chunks quickly (4 k-pairs), so
            # the ~1.5us eviction chain needs extra slack to stay hidden
            term_pass("w1pre", KC1, hx, ev_a, w0_tile=w0, defer=4)
            finale(red, inv1, None)

            # ---- t0, n2 / n3: single pre-term + const.
            # t1-n1's post/self term passes are wedged between them: they
            # only need s2(t0)/s1(t0) and don't touch comb (the t0 updates
            # don't use it), so their matmuls fill t0's serial-chain tails.
            def ev_t0(red, s8, inv_t, cidx, bpre):
                def ev(mc, rh, ps):
                    e2 = inv_mul(ps, inv_t, rh)
                    e = epool.tile([P, RH], F32, tag="e")
                    nc.scalar.activation(
                        e[:], e2[:], mybir.ActivationFunctionType.Relu,
                        bias=bias_ap(bpre, mc))
                    nc.vector.tensor_scalar_add(
                        snew[:, mc, rsl(rh)], e[:], bias_ap(cidx, mc))
                    if rh == 1:
                        store_sq_reduce(mc, s8, red)
                return ev

            red = red_pair()
            term_pass("w2pre", KC, s1, ev_t0(red, s2, inv1, C2, B2PRE))
            finale(red, inv2, None)

            term_pass("w1post", KC, s2, evict_to(comb, B1POST, inv2))
            term_pass("w1self", KC, s1, evict_add_comb(B1SELF, inv1))

            red = red_pair()
            term_pass("w3pre", KC, s2, ev_t0(red, s3, inv2, C3, B3PRE))
            finale(red, inv3, None)

            def n1_wedge(red):
                # n1 = A + relu(s2@w1post'+b) + relu(s1@w1self'+b): comb is
                # complete, so each mc chunk is pure DVE/ACT work; wedging it
                # into the w2post pass keeps the PE streaming.
                def wg(mc):
                    nc.vector.tensor_add(snew[:, mc, :], At[:, mc, :],
                                         comb[:, mc, :])
                    store_sq_reduce(mc, s1, red)
                return wg

            # ---- t1 / t2
            for t in (1, 2):
                last = (t == 2)
                if t == 2:
                    term_pass("w1post", KC, s2, evict_to(comb, B1POST, inv2))
                    term_pass("w1self", KC, s1, evict_add_comb(B1SELF, inv1))

                # n2 = relu(s1new@w2pre') + relu(s3@w2post') + relu(s2@w2self')
                red_n1 = red_pair()
                term_pass("w2post", KC, s3, evict_to(comb, B2POST, inv3),
                          wedge=n1_wedge(red_n1))
                finale(red_n1, inv1, "init" if last else None)
                term_pass("w2self", KC, s2, evict_add_comb(B2SELF, inv2))
                red = red_pair()

                def ev_n2(mc, rh, ps, red=red):
                    e2 = inv_mul(ps, inv1, rh)
                    e = epool.tile([P, RH], F32, tag="e")
                    nc.scalar.activation(
                        e[:], e2[:], mybir.ActivationFunctionType.Relu,
                        bias=bias_ap(B2PRE, mc))
                    nc.vector.tensor_add(snew[:, mc, rsl(rh)],
                                         e[:], comb[:, mc, rsl(rh)])
                    if rh == 1:
                        store_sq_reduce(mc, s2, red)

                term_pass("w2pre", KC, s1, ev_n2)
                finale(red, inv2, "add" if last else None)

                # n3 = relu(s2new@w3pre') + c3p + relu(s3@w3self')
                term_pass("w3self", KC, s3, evict_to(comb, B3SELF, inv3))
                red = red_pair()

                def ev_n3(mc, rh, ps, red=red):
                    e2 = inv_mul(ps, inv2, rh)
                    e = epool.tile([P, RH], F32, tag="e")
                    nc.scalar.activation(
                        e[:], e2[:], mybir.ActivationFunctionType.Relu,
                        bias=bias_ap(B3PRE, mc))
                    nc.vector.scalar_tensor_tensor(
                        snew[:, mc, rsl(rh)], e[:], bias_ap(C3P, mc),
                        comb[:, mc, rsl(rh)],
                        op0=mybir.AluOpType.add, op1=mybir.AluOpType.add)
                    if rh == 1:
                        store_sq_reduce(mc, s3, red)

                term_pass("w3pre", KC, s2, ev_n3)
                finale(red, None if last else inv3, "add" if last else None)

            # ---- goodness out: g = gacc / (2048 * G^2)
            gout = consts.tile([1, R], F32, tag="gout")
            nc.scalar.mul(gout[:], gacc[:], 1.0 / (H * G * G))
            nc.sync.dma_start(out=g_d[:], in_=gout[:])

    nc.compile()
    return nc


def _block_weight(w, scale, kcn):
    """[2048, d_in] float32 -> [MC, P, kcn, P] fp8 blocked for linear DMA:
    host_w[mc, p, kc, m] = scale * W[mc*128+m, kc*128+p]."""
    w = np.asarray(w, dtype=np.float32) * scale
    din = w.shape[1]
    if din < kcn * P:
        w = np.pad(w, ((0, 0), (0, kcn * P - din)))
    blk = w.reshape(MC, P, kcn, P).transpose(0, 3, 2, 1)
    return np.ascontiguousarray(np.clip(blk, -240.0, 240.0).astype(NPF8))


def _col(v):
    """[2048] -> [128, 16] (partition-major bias layout)."""
    return np.asarray(v, dtype=np.float32).reshape(MC, P).T


def prepare_inputs(inputs):
    """Host prep: overlay+normalize Hx, prescale/block weights, pack biases.
    Returns (shared_map, per_core_hx list)."""
    x = np.asarray(inputs["x"], dtype=np.float32)
    mx = x.max()
    base = x.copy()
    base[:, :NL] = 0.0
    hx = np.tile(base[None, :, :], (NL, 1, 1))
    for l in range(NL):
        hx[l, :, l] = mx
    hx = hx.reshape(ROWS, D_IN)
    n = np.linalg.norm(hx, axis=1, keepdims=True)
    hxn = (G / (n + EPS)) * hx
    hxn = np.pad(hxn, ((0, 0), (0, D_IN_PAD - D_IN)))

    per_core_hx = []
    for c in range(N_CORES):
        h = hxn[c * R:(c + 1) * R].T            # [1024, 640]
        h = h.reshape(KC1, P, R).transpose(1, 0, 2)
        per_core_hx.append(np.ascontiguousarray(
            np.clip(h, -240.0, 240.0).astype(NPF8)))

    wa = ALPHA
    shared = {
        "w1pre": _block_weight(inputs["w1_pre"], 0.7 * wa, KC1),
        "w1post": _block_weight(inputs["w1_post"], 0.7 * wa, KC),
        "w1self": _block_weight(inputs["w1_self"], 0.3 * wa, KC),
        "w2pre": _block_weight(inputs["w2_pre"], 0.7 * wa, KC),
        "w2post": _block_weight(inputs["w2_post"], 0.7 * wa, KC),
        "w2self": _block_weight(inputs["w2_self"], 0.3 * wa, KC),
        "w3pre": _block_weight(inputs["w3_pre"], 0.7 * wa, KC),
        "w3self": _block_weight(inputs["w3_self"], 0.3 * wa, KC),
    }

    relu = lambda a: np.maximum(np.asarray(a, dtype=np.float32), 0.0)

    cols = np.empty((P, NBIAS * MC), dtype=np.float32)
    vals = {
        B1PRE: G * 0.7 * np.asarray(inputs["b1_pre"], np.float32),
        B1POST: G * 0.7 * np.asarray(inputs["b1_post"], np.float32),
        B1SELF: G * 0.3 * np.asarray(inputs["b1_self"], np.float32),
        B2PRE: G * 0.7 * np.asarray(inputs["b2_pre"], np.float32),
        B2POST: G * 0.7 * np.asarray(inputs["b2_post"], np.float32),
        B2SELF: G * 0.3 * np.asarray(inputs["b2_self"], np.float32),
        B3PRE: G * 0.7 * np.asarray(inputs["b3_pre"], np.float32),
        B3SELF: G * 0.3 * np.asarray(inputs["b3_self"], np.float32),
        C1: G * (0.7 * relu(inputs["b1_post"]) + 0.3 * relu(inputs["b1_self"])),
        C2: G * (0.7 * relu(inputs["b2_post"]) + 0.3 * relu(inputs["b2_self"])),
        C3: G * (0.7 * relu(inputs["b3_post"]) + 0.3 * relu(inputs["b3_self"])),
        C3P: G * 0.7 * relu(inputs["b3_post"]),
    }
    for idx, v in vals.items():
        cols[:, idx * MC:(idx + 1) * MC] = _col(v)
    shared["biases"] = np.ascontiguousarray(cols)

    return shared, per_core_hx


def run(inputs, trace=False):
    shared, per_core_hx = prepare_inputs(inputs)
    if "nc" not in _NC_CACHE:
        _NC_CACHE["nc"] = _build_nc()
    nc = _NC_CACHE["nc"]
    in_maps = [dict(shared, hxn=per_core_hx[c]) for c in range(N_CORES)]
    res = run_bass_kernel_spmd(nc, in_maps, core_ids=list(range(N_CORES)),
                               trace=trace)
    g = np.concatenate([res.results[c]["g"][0] for c in range(N_CORES)])
    out = g.reshape(NL, B).T.astype(np.float32)
    return np.ascontiguousarray(out), res


def kernel(**inputs):
    out, _ = run(inputs, trace=False)
    return out


# revision 6
# speedup vs baseline: 1.9526x; 1.0059x over previous
"""Trainium2 Bass kernel for the 3-metalayer forward-forward style MLP.

Distribution: the (10 labels x 512 batch) grid flattens to 5120 independent
rows; each of the 8 cores processes 640 rows (pure data parallelism, weights
replicated, no collectives).

Device-side algorithm (per core, rows R=640):
  - matmul inputs (states, overlay input, weights) are fp8 e4m3; every
    linear term runs as DoubleRow matmuls (K=256 per instruction, 2 fp8
    weights per PE cell -> 2x MACs/cycle).
  - the working state snew is bf16 at gain G=64; its fp8 copy s_i (made by
    an ACT dtype-convert) feeds the matmuls. The row normalization is
    applied to the matmul OUTPUT (inv is per-row, so it commutes through
    the matmul): a DVE psum*inv multiply feeds the ACT relu+bias eviction.
    This removes the normalize->matmul serial dependency between passes.
    Norms/goodness come from the bf16 snew, so fp8 quantization noise does
    not bias the normalization (no coherent rho^2/2 shrink).
  - psum tiles are bank PAIRS [128, 2, 512] (one bank per row-half), so
    each eviction step is a single wide [128, 2x320] op - halving the
    per-op overhead on DVE/ACT and deepening the psum pipeline to 3 mc
    blocks.
  - weights carry gain ALPHA=1024 and the 0.7/0.3 metalayer blend (relu
    positive homogeneity); inv = (G/ALPHA)/(sqrt(ss)+G*eps) folds all
    gains; goodness = ss/(2048*G^2).
  - row L2 norms: square (DVE, fp8 out) + fp8 DoubleRow ones-matmul
    reduction over partition pairs (8 matmuls per 2048 features, M=128
    broadcasts the sum to every partition for free).
  - pass order puts the self-terms (older inv) before the post-terms
    (fresher inv) when initializing the comb accumulator, so evictions
    never wait on a just-computed inv; the t1/t2 n1 combine (pure DVE/ACT
    work) is wedged mc-by-mc into the following w2self pass so the PE
    never idles on it.
"""

import numpy as np
import ml_dtypes

import concourse.bass as bass
import concourse.tile as tile
from concourse import bacc, mybir
from concourse.bass_utils import run_bass_kernel_spmd

BF = mybir.dt.bfloat16
F32 = mybir.dt.float32
FP8 = mybir.dt.float8e4
NPBF = ml_dtypes.bfloat16
NPF8 = ml_dtypes.float8_e4m3
DR = mybir.MatmulPerfMode.DoubleRow

N_CORES = 8
P = 128
D_IN = 784
D_IN_PAD = 1024           # 8 * 128 (even k-chunk count for DoubleRow pairs)
KC1 = 8                   # k-chunks for the 784->2048 matmul (padded)
KC = 16                   # k-chunks for 2048-contraction matmuls
MC = 16                   # output-feature chunks (2048 / 128)
H = 2048
B = 512
NL = 10
ROWS = NL * B             # 5120
R = ROWS // N_CORES       # 640 rows per core
RH = 320                  # psum row-chunk (one bank per row-half)
BK = 512                  # f32 elems per psum bank
EPS = 1e-4

ALPHA = 1024.0            # weight fp8 gain
G = 64.0                  # state gain

# bias/const column indices inside the packed [128, 12*16] bias tensor
B1PRE, B1POST, B1SELF, B2PRE, B2POST, B2SELF, B3PRE, B3SELF, C1, C2, C3, C3P = range(12)
NBIAS = 12

_NC_CACHE = {}


def _build_nc():
    """Build the single-core Tile program (same NEFF for all 8 cores)."""
    nc = bacc.Bacc("TRN2", target_bir_lowering=False, debug=False,
                   num_devices=N_CORES)

    hx_d = nc.dram_tensor("hxn", [P, KC1, R], FP8, kind="ExternalInput")
    w_d = {
        "w1pre": nc.dram_tensor("w1pre", [MC, P, KC1, P], FP8, kind="ExternalInput"),
    }
    for name in ("w1post", "w1self", "w2pre", "w2post", "w2self", "w3pre", "w3self"):
        w_d[name] = nc.dram_tensor(name, [MC, P, KC, P], FP8, kind="ExternalInput")
    bias_d = nc.dram_tensor("biases", [P, NBIAS * MC], F32, kind="ExternalInput")
    g_d = nc.dram_tensor("g", [1, R], F32, kind="ExternalOutput")

    with tile.TileContext(nc) as tc:
        with (
            tc.tile_pool(name="consts", bufs=1) as consts,
            tc.tile_pool(name="states", bufs=1) as states,
            tc.tile_pool(name="wpool", bufs=12) as wpool,
            tc.tile_pool(name="epool", bufs=6) as epool,
            tc.tile_pool(name="sqpool", bufs=4) as sqpool,
            tc.tile_pool(name="small", bufs=2) as small,
            tc.tile_pool(name="mmps", bufs=3, space="PSUM") as mmps,
            tc.tile_pool(name="redps", bufs=1, space="PSUM") as redps,
        ):
            # startup order: first hx chunk + first weight block must land
            # before anything else so the PE starts within ~1.5us
            hx = states.tile([P, KC1, R], FP8, tag="hxn")
            nc.sync.dma_start(out=hx[:, 0:2, :], in_=hx_d[:, 0:2, :])
            bias_sb = consts.tile([P, NBIAS * MC], F32)
            w0 = wpool.tile([P, KC1, P], FP8, tag="w", name="w1pre0")
            nc.sync.dma_start(out=w0[:], in_=w_d["w1pre"][0])
            nc.sync.dma_start(out=bias_sb[:], in_=bias_d[:])
            nc.sync.dma_start(out=hx[:, 2:KC1, :], in_=hx_d[:, 2:KC1, :])
            # [128, 2, 128] fp8 ones: M=128 DoubleRow ones-matmul reduces a
            # partition-pair AND broadcasts the row sum to every partition
            ones8 = consts.tile([P, 2, P], FP8)
            nc.vector.memset(ones8[:], 1.0)
            gacc = consts.tile([1, R], F32)

            # warm the PE HAM clock gate on the first hx chunk (lands ~3us,
            # well before the DVE boot path): >3.4us of dummy matmuls so the
            # real stream starts at 2.4GHz instead of 1.2GHz
            warm_ps = mmps.tile([P, 2, BK], F32, tag="mm", name="warm_ps")
            for _ in range(56):
                nc.tensor.matmul(warm_ps[:, 0, :P], hx[:, 0, 0:P],
                                 hx[:, 0, 0:P], start=True, stop=True)
            At = states.tile([P, MC, R], BF, tag="A")
            snew = states.tile([P, MC, R], BF, tag="snew")
            s1 = states.tile([P, MC, R], FP8, tag="s1")
            s2 = states.tile([P, MC, R], FP8, tag="s2")
            s3 = states.tile([P, MC, R], FP8, tag="s3")
            comb = states.tile([P, MC, R], BF, tag="comb")
            inv1 = states.tile([P, 2, RH], F32, tag="inv1")
            inv2 = states.tile([P, 2, RH], F32, tag="inv2")
            inv3 = states.tile([P, 2, RH], F32, tag="inv3")

            _red_uid = [0]

            def red_tile():
                _red_uid[0] += 1
                return redps.tile([P, 2, BK], F32, tag="red",
                                  name=f"red{_red_uid[0]}")

            def bias_ap(idx, mc):
                col = idx * MC + mc
                return bias_sb[:, col:col + 1]

            def rsl(rh):
                return slice(rh * RH, (rh + 1) * RH)

            def term_pass(wname, kcn, src, evict, w0_tile=None, defer=2,
                          wedge=None):
                """One linear term: stream weight blocks, accumulate psum
                bank-pairs, hand each [128, 2, 320] pair to `evict(mc, pst)`.

                DoubleRow: each matmul consumes a k-pair (K=256); the two
                row-halves run back-to-back on the same stationary weights.

                Evictions are emitted `defer` mc blocks late: the eviction
                chain (DVE inv-mul -> ACT relu -> DVE combine) has ~2us of
                cross-engine latency, and emitting it inline makes the
                strict-FIFO PE queue stall. Deferring places it behind
                independent matmul work.

                `wedge(mc)`, if given, emits independent non-PE work after
                each mc block's matmuls (used to hide the n1 combine)."""
                wd = w_d[wname]
                kpn = kcn // 2
                pending = []
                for mc in range(MC):
                    if mc == 0 and w0_tile is not None:
                        wt = w0_tile
                    else:
                        wt = wpool.tile([P, kcn, P], FP8, tag="w")
                        nc.sync.dma_start(out=wt[:], in_=wd[mc])
                    pst = mmps.tile([P, 2, BK], F32, tag="mm",
                                    name=f"mm_{wname}_{mc}")
                    for kp in range(kpn):
                        for rh in range(2):
                            nc.tensor.matmul(
                                pst[:, rh, 0:RH], wt[:, 2 * kp:2 * kp + 2, :],
                                src[:, 2 * kp:2 * kp + 2, rsl(rh)],
                                start=(kp == 0), stop=(kp == kpn - 1),
                                perf_mode=DR)
                    if wedge is not None:
                        wedge(mc)
                    pending.append((mc, pst))
                    if len(pending) > defer:
                        evict(*pending.pop(0))
                while pending:
                    evict(*pending.pop(0))

            _sqt = [None]

            def store_sq_reduce(mc, s8, red, copy=True):
                """snew chunk mc is complete: make the fp8 matmul copy,
                square (fp8), and on odd mc reduce the pair into the red
                psum with a DoubleRow ones-matmul."""
                if copy:
                    nc.scalar.copy(s8[:, mc, :], snew[:, mc, :])
                if mc % 2 == 0:
                    _sqt[0] = sqpool.tile([P, 2, R], FP8, tag="sq",
                                          name=f"sq{mc}")
                sqt = _sqt[0]
                nc.vector.tensor_mul(sqt[:, mc % 2, :], snew[:, mc, :],
                                     snew[:, mc, :])
                if mc % 2 == 1:
                    pair = mc // 2
                    for rh in range(2):
                        nc.tensor.matmul(red[:, rh, 0:RH], ones8[:],
                                         sqt[:, :, rsl(rh)],
                                         start=(pair == 0), stop=(pair == 7),
                                         perf_mode=DR)

            def finale(red, inv_t, goodness):
                """red holds ss = sum(snew^2) per row (gain G^2), already
                broadcast across all 128 partitions. inv_t (if wanted) gets
                (G/ALPHA)/(sqrt(ss)+G*eps), the per-row factor that turns
                a raw psum into the normalized-input result."""
                if goodness:
                    for rh in range(2):
                        if goodness == "init":
                            nc.vector.tensor_copy(gacc[:, rsl(rh)],
                                                  red[0:1, rh, 0:RH])
                        else:
                            nc.vector.tensor_add(gacc[:, rsl(rh)],
                                                 gacc[:, rsl(rh)],
                                                 red[0:1, rh, 0:RH])
                if inv_t is None:
                    return
                nr = small.tile([P, 2, RH], F32, tag="nr")
                nc.scalar.activation(nr[:], red[:, :, 0:RH],
                                     mybir.ActivationFunctionType.Sqrt,
                                     scale=(ALPHA / G) ** 2)
                nc.vector.tensor_scalar_add(nr[:], nr[:], ALPHA * EPS)
                nc.vector.reciprocal_approx_fast(out=inv_t[:], in_=nr[:])

            def inv_mul(pst, inv_t):
                e2 = epool.tile([P, 2, RH], F32, tag="e2")
                nc.vector.tensor_mul(e2[:], pst[:, :, 0:RH], inv_t[:])
                return e2

            def evict_to(dst, bidx, inv_t):
                def ev(mc, pst):
                    e2 = inv_mul(pst, inv_t)
                    nc.scalar.activation(
                        dst[:, mc, :], e2[:],
                        mybir.ActivationFunctionType.Relu,
                        bias=bias_ap(bidx, mc))
                return ev

            def evict_add_comb(bidx, inv_t):
                def ev(mc, pst):
                    e2 = inv_mul(pst, inv_t)
                    e = epool.tile([P, 2, RH], F32, tag="e")
                    nc.scalar.activation(
                        e[:], e2[:], mybir.ActivationFunctionType.Relu,
                        bias=bias_ap(bidx, mc))
                    nc.vector.tensor_add(comb[:, mc, :], e[:], comb[:, mc, :])
                return ev

            # ---- A = relu((hxn @ w1pre')/ALPHA + G*0.7*b1pre), cached for
            # all steps (hx is host-prenormalized: no inv needed).
            # t0-n1 (snew = A + c1) is fused into the same pass.
            red = red_tile()

            def ev_a(mc, pst, red=red):
                nc.scalar.activation(
                    At[:, mc, :], pst[:, :, 0:RH],
                    mybir.ActivationFunctionType.Relu,
                    bias=bias_ap(B1PRE, mc), scale=1.0 / ALPHA)
                nc.vector.tensor_scalar_add(
                    snew[:, mc, :], At[:, mc, :], bias_ap(C1, mc))
                store_sq_reduce(mc, s1, red)

            term_pass("w1pre", KC1, hx, ev_a, w0_tile=w0)
            finale(red, inv1, None)

            # ---- t0, n2 / n3: single pre-term + const.
            # t1-n1's self/post term passes are wedged between them: they
            # only need s1(t0)/s2(t0) and don't touch comb (the t0 updates
            # don't use it), so their matmuls fill t0's serial-chain tails.
            def ev_t0(red, s8, inv_t, cidx, bpre):
                def ev(mc, pst):
                    e2 = inv_mul(pst, inv_t)
                    e = epool.tile([P, 2, RH], F32, tag="e")
                    nc.scalar.activation(
                        e[:], e2[:], mybir.ActivationFunctionType.Relu,
                        bias=bias_ap(bpre, mc))
                    nc.vector.tensor_scalar_add(
                        snew[:, mc, :], e[:], bias_ap(cidx, mc))
                    store_sq_reduce(mc, s8, red)
                return ev

            red = red_tile()
            term_pass("w2pre", KC, s1, ev_t0(red, s2, inv1, C2, B2PRE))
            finale(red, inv2, None)

            # self-term first (its inv is a pass older), post-term second:
            # an eviction never waits on a just-finalized inv
            term_pass("w1self", KC, s1, evict_to(comb, B1SELF, inv1))
            term_pass("w1post", KC, s2, evict_add_comb(B1POST, inv2))

            red = red_tile()
            term_pass("w3pre", KC, s2, ev_t0(red, s3, inv2, C3, B3PRE))
            finale(red, inv3, None)

            def n1_wedge(red):
                # n1 = A + relu(s1@w1self'+b) + relu(s2@w1post'+b): comb is
                # complete, so each mc chunk is pure DVE/ACT work; wedging it
                # into the w2self pass keeps the PE streaming.
                def wg(mc):
                    nc.vector.tensor_add(snew[:, mc, :], At[:, mc, :],
                                         comb[:, mc, :])
                    store_sq_reduce(mc, s1, red)
                return wg

            # ---- t1 / t2
            for t in (1, 2):
                last = (t == 2)
                if t == 2:
                    term_pass("w1self", KC, s1, evict_to(comb, B1SELF, inv1))
                    term_pass("w1post", KC, s2, evict_add_comb(B1POST, inv2))

                # n2 = relu(s1new@w2pre') + relu(s2@w2self') + relu(s3@w2post')
                red_n1 = red_tile()
                term_pass("w2self", KC, s2, evict_to(comb, B2SELF, inv2),
                          wedge=n1_wedge(red_n1))
                finale(red_n1, inv1, "init" if last else None)
                term_pass("w2post", KC, s3, evict_add_comb(B2POST, inv3))
                red = red_tile()

                def ev_n2(mc, pst, red=red):
                    e2 = inv_mul(pst, inv1)
                    e = epool.tile([P, 2, RH], F32, tag="e")
                    nc.scalar.activation(
                        e[:], e2[:], mybir.ActivationFunctionType.Relu,
                        bias=bias_ap(B2PRE, mc))
                    nc.vector.tensor_add(snew[:, mc, :], e[:], comb[:, mc, :])
                    store_sq_reduce(mc, s2, red)

                term_pass("w2pre", KC, s1, ev_n2)
                finale(red, inv2, "add" if last else None)

                # n3 = relu(s2new@w3pre') + c3p + relu(s3@w3self')
                term_pass("w3self", KC, s3, evict_to(comb, B3SELF, inv3))
                red = red_tile()

                def ev_n3(mc, pst, red=red, last=last):
                    e2 = inv_mul(pst, inv2)
                    e = epool.tile([P, 2, RH], F32, tag="e")
                    nc.scalar.activation(
                        e[:], e2[:], mybir.ActivationFunctionType.Relu,
                        bias=bias_ap(B3PRE, mc))
                    nc.vector.scalar_tensor_tensor(
                        snew[:, mc, :], e[:], bias_ap(C3P, mc),
                        comb[:, mc, :],
                        op0=mybir.AluOpType.add, op1=mybir.AluOpType.add)
                    # s3(t2) is never consumed by a matmul: skip its fp8 copy
                    store_sq_reduce(mc, s3, red, copy=not last)

                term_pass("w3pre", KC, s2, ev_n3)
                finale(red, None if last else inv3, "add" if last else None)

            # ---- goodness out: g = gacc / (2048 * G^2)
            gout = consts.tile([1, R], F32, tag="gout")
            nc.scalar.mul(gout[:], gacc[:], 1.0 / (H * G * G))
            nc.sync.dma_start(out=g_d[:], in_=gout[:])

    nc.compile()
    return nc


def _block_weight(w, scale, kcn):
    """[2048, d_in] float32 -> [MC, P, kcn, P] fp8 blocked for linear DMA:
    host_w[mc, p, kc, m] = scale * W[mc*128+m, kc*128+p]."""
    w = np.asarray(w, dtype=np.float32) * scale
    din = w.shape[1]
    if din < kcn * P:
        w = np.pad(w, ((0, 0), (0, kcn * P - din)))
    blk = w.reshape(MC, P, kcn, P).transpose(0, 3, 2, 1)
    return np.ascontiguousarray(np.clip(blk, -240.0, 240.0).astype(NPF8))


def _col(v):
    """[2048] -> [128, 16] (partition-major bias layout)."""
    return np.asarray(v, dtype=np.float32).reshape(MC, P).T


def prepare_inputs(inputs):
    """Host prep: overlay+normalize Hx, prescale/block weights, pack biases.
    Returns (shared_map, per_core_hx list)."""
    x = np.asarray(inputs["x"], dtype=np.float32)
    mx = x.max()
    base = x.copy()
    base[:, :NL] = 0.0
    hx = np.tile(base[None, :, :], (NL, 1, 1))
    for l in range(NL):
        hx[l, :, l] = mx
    hx = hx.reshape(ROWS, D_IN)
    n = np.linalg.norm(hx, axis=1, keepdims=True)
    hxn = (G / (n + EPS)) * hx
    hxn = np.pad(hxn, ((0, 0), (0, D_IN_PAD - D_IN)))

    per_core_hx = []
    for c in range(N_CORES):
        h = hxn[c * R:(c + 1) * R].T            # [1024, 640]
        h = h.reshape(KC1, P, R).transpose(1, 0, 2)
        per_core_hx.append(np.ascontiguousarray(
            np.clip(h, -240.0, 240.0).astype(NPF8)))

    wa = ALPHA
    shared = {
        "w1pre": _block_weight(inputs["w1_pre"], 0.7 * wa, KC1),
        "w1post": _block_weight(inputs["w1_post"], 0.7 * wa, KC),
        "w1self": _block_weight(inputs["w1_self"], 0.3 * wa, KC),
        "w2pre": _block_weight(inputs["w2_pre"], 0.7 * wa, KC),
        "w2post": _block_weight(inputs["w2_post"], 0.7 * wa, KC),
        "w2self": _block_weight(inputs["w2_self"], 0.3 * wa, KC),
        "w3pre": _block_weight(inputs["w3_pre"], 0.7 * wa, KC),
        "w3self": _block_weight(inputs["w3_self"], 0.3 * wa, KC),
    }

    relu = lambda a: np.maximum(np.asarray(a, dtype=np.float32), 0.0)

    cols = np.empty((P, NBIAS * MC), dtype=np.float32)
    vals = {
        B1PRE: G * 0.7 * np.asarray(inputs["b1_pre"], np.float32),
        B1POST: G * 0.7 * np.asarray(inputs["b1_post"], np.float32),
        B1SELF: G * 0.3 * np.asarray(inputs["b1_self"], np.float32),
        B2PRE: G * 0.7 * np.asarray(inputs["b2_pre"], np.float32),
        B2POST: G * 0.7 * np.asarray(inputs["b2_post"], np.float32),
        B2SELF: G * 0.3 * np.asarray(inputs["b2_self"], np.float32),
        B3PRE: G * 0.7 * np.asarray(inputs["b3_pre"], np.float32),
        B3SELF: G * 0.3 * np.asarray(inputs["b3_self"], np.float32),
        C1: G * (0.7 * relu(inputs["b1_post"]) + 0.3 * relu(inputs["b1_self"])),
        C2: G * (0.7 * relu(inputs["b2_post"]) + 0.3 * relu(inputs["b2_self"])),
        C3: G * (0.7 * relu(inputs["b3_post"]) + 0.3 * relu(inputs["b3_self"])),
        C3P: G * 0.7 * relu(inputs["b3_post"]),
    }
    for idx, v in vals.items():
        cols[:, idx * MC:(idx + 1) * MC] = _col(v)
    shared["biases"] = np.ascontiguousarray(cols)

    return shared, per_core_hx


def run(inputs, trace=False):
    shared, per_core_hx = prepare_inputs(inputs)
    if "nc" not in _NC_CACHE:
        _NC_CACHE["nc"] = _build_nc()
    nc = _NC_CACHE["nc"]
    in_maps = [dict(shared, hxn=per_core_hx[c]) for c in range(N_CORES)]
    res = run_bass_kernel_spmd(nc, in_maps, core_ids=list(range(N_CORES)),
                               trace=trace)
    g = np.concatenate([res.results[c]["g"][0] for c in range(N_CORES)])
    out = g.reshape(NL, B).T.astype(np.float32)
    return np.ascontiguousarray(out), res


def kernel(**inputs):
    out, _ = run(inputs, trace=False)
    return out


# revision 16
# speedup vs baseline: 1.9773x; 1.0126x over previous
"""Trainium2 Bass kernel for the 3-metalayer forward-forward style MLP.

Distribution: the (10 labels x 512 batch) grid flattens to 5120 independent
rows; each of the 8 cores processes 640 rows (pure data parallelism, weights
replicated, no collectives).

Device-side algorithm (per core, rows R=640):
  - matmul inputs (states, overlay input, weights) are fp8 e4m3; every
    linear term runs as DoubleRow matmuls (K=256 per instruction, 2 fp8
    weights per PE cell -> 2x MACs/cycle).
  - the working state snew is bf16 at gain G=64; its fp8 copy s_i (made by
    an ACT dtype-convert) feeds the matmuls. The row normalization is
    applied to the matmul OUTPUT (inv is per-row, so it commutes through
    the matmul): a DVE psum*inv multiply feeds the ACT relu+bias eviction.
    This removes the normalize->matmul serial dependency between passes.
    Norms/goodness come from the bf16 snew, so fp8 quantization noise does
    not bias the normalization (no coherent rho^2/2 shrink).
  - psum tiles are bank PAIRS [128, 2, 512] (one bank per row-half), so
    each eviction step is a single wide [128, 2x320] op - halving the
    per-op overhead on DVE/ACT and deepening the psum pipeline to 3 mc
    blocks.
  - weights carry gain ALPHA=1024 and the 0.7/0.3 metalayer blend (relu
    positive homogeneity); inv = (G/ALPHA)/(sqrt(ss)+G*eps) folds all
    gains; goodness = ss/(2048*G^2).
  - row L2 norms: square (DVE, fp8 out) + fp8 DoubleRow ones-matmul
    reduction over partition pairs (8 matmuls per 2048 features, M=128
    broadcasts the sum to every partition for free).
  - pass order puts the self-terms (older inv) before the post-terms
    (fresher inv) when initializing the comb accumulator, so evictions
    never wait on a just-computed inv; the t1/t2 n1 combine (pure DVE/ACT
    work) is wedged mc-by-mc into the following w2self pass so the PE
    never idles on it.
"""

import numpy as np
import ml_dtypes

import concourse.bass as bass
import concourse.tile as tile
from concourse import bacc, mybir
from concourse.bass_utils import run_bass_kernel_spmd

BF = mybir.dt.bfloat16
F32 = mybir.dt.float32
FP8 = mybir.dt.float8e4
NPBF = ml_dtypes.bfloat16
NPF8 = ml_dtypes.float8_e4m3
DR = mybir.MatmulPerfMode.DoubleRowSwInterleave

N_CORES = 8
P = 128
D_IN = 784
D_IN_PAD = 1024           # 8 * 128 (even k-chunk count for DoubleRow pairs)
KC1 = 8                   # k-chunks for the 784->2048 matmul (padded)
KC = 16                   # k-chunks for 2048-contraction matmuls
MC = 16                   # output-feature chunks (2048 / 128)
H = 2048
B = 512
NL = 10
ROWS = NL * B             # 5120
R = ROWS // N_CORES       # 640 rows per core
RH = 320                  # psum row-chunk (one bank per row-half)
BK = 512                  # f32 elems per psum bank
EPS = 1e-4

ALPHA = 1024.0            # weight fp8 gain
G = 64.0                  # state gain

# bias/const column indices inside the packed [128, 12*16] bias tensor
B1PRE, B1POST, B1SELF, B2PRE, B2POST, B2SELF, B3PRE, B3SELF, C1, C2, C3, C3P = range(12)
NBIAS = 12

_NC_CACHE = {}


def _build_nc():
    """Build the single-core Tile program (same NEFF for all 8 cores)."""
    nc = bacc.Bacc("TRN2", target_bir_lowering=False, debug=False,
                   num_devices=N_CORES)

    hx_d = nc.dram_tensor("hxn", [P, KC1, R], FP8, kind="ExternalInput")
    w_d = {
        "w1pre": nc.dram_tensor("w1pre", [MC, P, KC1 // 2, 2, P], FP8,
                                kind="ExternalInput"),
    }
    for name in ("w1post", "w1self", "w2pre", "w2post", "w2self", "w3pre", "w3self"):
        w_d[name] = nc.dram_tensor(name, [MC, P, KC // 2, 2, P], FP8,
                                   kind="ExternalInput")
    bias_d = nc.dram_tensor("biases", [P, NBIAS * MC], F32, kind="ExternalInput")
    g_d = nc.dram_tensor("g", [1, R], F32, kind="ExternalOutput")

    with tile.TileContext(nc) as tc:
        with (
            tc.tile_pool(name="consts", bufs=1) as consts,
            tc.tile_pool(name="states", bufs=1) as states,
            tc.tile_pool(name="wpool", bufs=12) as wpool,
            tc.tile_pool(name="epool", bufs=6) as epool,
            tc.tile_pool(name="sqpool", bufs=4) as sqpool,
            tc.tile_pool(name="small", bufs=2) as small,
            tc.tile_pool(name="mmps", bufs=3, space="PSUM") as mmps,
            tc.tile_pool(name="redps", bufs=1, space="PSUM") as redps,
        ):
            # startup order: first hx chunk + first weight block must land
            # before anything else so the PE starts within ~1.5us
            hx = states.tile([P, KC1, R], FP8, tag="hxn")
            nc.sync.dma_start(out=hx[:, 0:2, :], in_=hx_d[:, 0:2, :])
            bias_sb = consts.tile([P, NBIAS * MC], F32)
            w0 = wpool.tile([P, KC1 // 2, 2, P], FP8, tag="w", name="w1pre0")
            nc.sync.dma_start(out=w0[:], in_=w_d["w1pre"][0])
            nc.sync.dma_start(out=bias_sb[:], in_=bias_d[:])
            nc.sync.dma_start(out=hx[:, 2:KC1, :], in_=hx_d[:, 2:KC1, :])
            # [128, 2, 128] fp8 ones: M=128 DoubleRow ones-matmul reduces a
            # partition-pair AND broadcasts the row sum to every partition
            ones8 = consts.tile([P, 2, P], FP8)
            nc.vector.memset(ones8[:], 1.0)
            gacc = consts.tile([1, R], F32)

            # warm the PE HAM clock gate with matmuls on a junk tile
            # (contents irrelevant, psum never read): written by the
            # otherwise-idle GPSIMD engine so the PE starts right after its
            # boot sequence, and >3.4us of activity ramps the clock to
            # 2.4GHz before real work
            junk = consts.tile([P, P], FP8, tag="junk")
            nc.gpsimd.memset(junk[:], 1.0)
            warm_ps = mmps.tile([P, 2, BK], F32, tag="mm", name="warm_ps")
            for _ in range(48):
                nc.tensor.matmul(warm_ps[:, 0, :P], junk[:], junk[:],
                                 start=True, stop=True)
            At = states.tile([P, MC, R], BF, tag="A")
            snew = states.tile([P, MC, R], BF, tag="snew")
            s1 = states.tile([P, MC, R], FP8, tag="s1")
            s2 = states.tile([P, MC, R], FP8, tag="s2")
            s3 = states.tile([P, MC, R], FP8, tag="s3")
            comb = states.tile([P, MC, R], BF, tag="comb")
            inv1 = states.tile([P, 2, RH], F32, tag="inv1")
            inv2 = states.tile([P, 2, RH], F32, tag="inv2")
            inv3 = states.tile([P, 2, RH], F32, tag="inv3")

            _red_uid = [0]

            def red_tile():
                _red_uid[0] += 1
                return redps.tile([P, 2, BK], F32, tag="red",
                                  name=f"red{_red_uid[0]}")

            def bias_ap(idx, mc):
                col = idx * MC + mc
                return bias_sb[:, col:col + 1]

            def rsl(rh):
                return slice(rh * RH, (rh + 1) * RH)

            def term_pass(wname, kcn, src, evict, w0_tile=None, defer=2,
                          wedge=None):
                """One linear term: stream weight blocks, accumulate psum
                bank-pairs, hand each [128, 2, 320] pair to `evict(mc, pst)`.

                DoubleRow: each matmul consumes a k-pair (K=256); the two
                row-halves run back-to-back on the same stationary weights.

                Evictions are emitted `defer` mc blocks late: the eviction
                chain (DVE inv-mul -> ACT relu -> DVE combine) has ~2us of
                cross-engine latency, and emitting it inline makes the
                strict-FIFO PE queue stall. Deferring places it behind
                independent matmul work.

                `wedge(mc)`, if given, emits independent non-PE work after
                each mc block's matmuls (used to hide the n1 combine)."""
                wd = w_d[wname]
                kpn = kcn // 2
                pending = []
                for mc in range(MC):
                    if mc == 0 and w0_tile is not None:
                        wt = w0_tile
                    else:
                        wt = wpool.tile([P, kpn, 2, P], FP8, tag="w")
                        nc.sync.dma_start(out=wt[:], in_=wd[mc])
                    pst = mmps.tile([P, 2, BK], F32, tag="mm",
                                    name=f"mm_{wname}_{mc}")
                    for kp in range(kpn):
                        for rh in range(2):
                            nc.tensor.matmul(
                                pst[:, rh, 0:RH], wt[:, kp, :, :],
                                src[:, 2 * kp:2 * kp + 2, rsl(rh)],
                                start=(kp == 0), stop=(kp == kpn - 1),
                                perf_mode=DR)
                    if wedge is not None:
                        wedge(mc)
                    pending.append((mc, pst))
                    if len(pending) > defer:
                        evict(*pending.pop(0))
                while pending:
                    evict(*pending.pop(0))

            _sqt = [None]

            def store_sq_reduce(mc, s8, red, copy=True):
                """snew chunk mc is complete: make the fp8 matmul copy,
                square (fp8), and on odd mc reduce the pair into the red
                psum with a DoubleRow ones-matmul."""
                if copy:
                    nc.scalar.copy(s8[:, mc, :], snew[:, mc, :])
                if mc % 2 == 0:
                    _sqt[0] = sqpool.tile([P, 2, R], FP8, tag="sq",
                                          name=f"sq{mc}")
                sqt = _sqt[0]
                nc.vector.tensor_mul(sqt[:, mc % 2, :], snew[:, mc, :],
                                     snew[:, mc, :])
                if mc % 2 == 1:
                    pair = mc // 2
                    for rh in range(2):
                        nc.tensor.matmul(red[:, rh, 0:RH], ones8[:],
                                         sqt[:, :, rsl(rh)],
                                         start=(pair == 0), stop=(pair == 7),
                                         perf_mode=DR)

            def finale(red, inv_t, goodness):
                """red holds ss = sum(snew^2) per row (gain G^2), already
                broadcast across all 128 partitions. inv_t (if wanted) gets
                (G/ALPHA)/(sqrt(ss)+G*eps), the per-row factor that turns
                a raw psum into the normalized-input result."""
                if goodness:
                    for rh in range(2):
                        if goodness == "init":
                            nc.vector.tensor_copy(gacc[:, rsl(rh)],
                                                  red[0:1, rh, 0:RH])
                        else:
                            nc.vector.tensor_add(gacc[:, rsl(rh)],
                                                 gacc[:, rsl(rh)],
                                                 red[0:1, rh, 0:RH])
                if inv_t is None:
                    return
                nr = small.tile([P, 2, RH], F32, tag="nr")
                nc.scalar.activation(nr[:], red[:, :, 0:RH],
                                     mybir.ActivationFunctionType.Sqrt,
                                     scale=(ALPHA / G) ** 2)
                nc.vector.tensor_scalar_add(nr[:], nr[:], ALPHA * EPS)
                nc.vector.reciprocal_approx_fast(out=inv_t[:], in_=nr[:])

            def inv_mul(pst, inv_t):
                e2 = epool.tile([P, 2, RH], F32, tag="e2")
                nc.vector.tensor_mul(e2[:], pst[:, :, 0:RH], inv_t[:])
                return e2

            def evict_to(dst, bidx, inv_t):
                def ev(mc, pst):
                    e2 = inv_mul(pst, inv_t)
                    nc.scalar.activation(
                        dst[:, mc, :], e2[:],
                        mybir.ActivationFunctionType.Relu,
                        bias=bias_ap(bidx, mc))
                return ev

            def evict_add_comb(bidx, inv_t):
                def ev(mc, pst):
                    e2 = inv_mul(pst, inv_t)
                    e = epool.tile([P, 2, RH], F32, tag="e")
                    nc.scalar.activation(
                        e[:], e2[:], mybir.ActivationFunctionType.Relu,
                        bias=bias_ap(bidx, mc))
                    nc.vector.tensor_add(comb[:, mc, :], e[:], comb[:, mc, :])
                return ev

            # ---- A = relu((hxn @ w1pre')/ALPHA + G*0.7*b1pre), cached for
            # all steps (hx is host-prenormalized: no inv needed).
            # t0-n1 (snew = A + c1) is fused into the same pass.
            red = red_tile()

            def ev_a(mc, pst, red=red):
                nc.scalar.activation(
                    At[:, mc, :], pst[:, :, 0:RH],
                    mybir.ActivationFunctionType.Relu,
                    bias=bias_ap(B1PRE, mc), scale=1.0 / ALPHA)
                nc.vector.tensor_scalar_add(
                    snew[:, mc, :], At[:, mc, :], bias_ap(C1, mc))
                store_sq_reduce(mc, s1, red)

            term_pass("w1pre", KC1, hx, ev_a, w0_tile=w0)
            finale(red, inv1, None)

            # ---- t0, n2 / n3: single pre-term + const.
            # t1-n1's self/post term passes are wedged between them: they
            # only need s1(t0)/s2(t0) and don't touch comb (the t0 updates
            # don't use it), so their matmuls fill t0's serial-chain tails.
            def ev_t0(red, s8, inv_t, cidx, bpre):
                def ev(mc, pst):
                    e2 = inv_mul(pst, inv_t)
                    e = epool.tile([P, 2, RH], F32, tag="e")
                    nc.scalar.activation(
                        e[:], e2[:], mybir.ActivationFunctionType.Relu,
                        bias=bias_ap(bpre, mc))
                    nc.vector.tensor_scalar_add(
                        snew[:, mc, :], e[:], bias_ap(cidx, mc))
                    store_sq_reduce(mc, s8, red)
                return ev

            red = red_tile()
            term_pass("w2pre", KC, s1, ev_t0(red, s2, inv1, C2, B2PRE))
            finale(red, inv2, None)

            # self-term first (its inv is a pass older), post-term second:
            # an eviction never waits on a just-finalized inv
            term_pass("w1self", KC, s1, evict_to(comb, B1SELF, inv1))
            term_pass("w1post", KC, s2, evict_add_comb(B1POST, inv2))

            red = red_tile()
            term_pass("w3pre", KC, s2, ev_t0(red, s3, inv2, C3, B3PRE))
            finale(red, inv3, None)

            def n1_wedge(red):
                # n1 = A + relu(s1@w1self'+b) + relu(s2@w1post'+b): comb is
                # complete, so each mc chunk is pure DVE/ACT work; wedging it
                # into the w2self pass keeps the PE streaming. Lag by 2 mc
                # blocks so the wedge's reduce-matmuls queue well behind
                # their DVE producers (no PE stall on the n1 chain).
                def unit(mc):
                    nc.vector.tensor_add(snew[:, mc, :], At[:, mc, :],
                                         comb[:, mc, :])
                    store_sq_reduce(mc, s1, red)

                def wg(mc):
                    if mc >= 2:
                        unit(mc - 2)
                    if mc == MC - 1:
                        unit(MC - 2)
                        unit(MC - 1)
                return wg

            # ---- t1 / t2
            for t in (1, 2):
                last = (t == 2)
                if t == 2:
                    term_pass("w1self", KC, s1, evict_to(comb, B1SELF, inv1))
                    term_pass("w1post", KC, s2, evict_add_comb(B1POST, inv2))

                # n2 = relu(s1new@w2pre') + relu(s2@w2self') + relu(s3@w2post')
                red_n1 = red_tile()
                term_pass("w2self", KC, s2, evict_to(comb, B2SELF, inv2),
                          wedge=n1_wedge(red_n1))
                finale(red_n1, inv1, "init" if last else None)
                term_pass("w2post", KC, s3, evict_add_comb(B2POST, inv3))
                red = red_tile()

                def ev_n2(mc, pst, red=red):
                    e2 = inv_mul(pst, inv1)
                    e = epool.tile([P, 2, RH], F32, tag="e")
                    nc.scalar.activation(
                        e[:], e2[:], mybir.ActivationFunctionType.Relu,
                        bias=bias_ap(B2PRE, mc))
                    nc.vector.tensor_add(snew[:, mc, :], e[:], comb[:, mc, :])
                    store_sq_reduce(mc, s2, red)

                term_pass("w2pre", KC, s1, ev_n2)
                finale(red, inv2, "add" if last else None)

                # n3 = relu(s2new@w3pre') + c3p + relu(s3@w3self')
                term_pass("w3self", KC, s3, evict_to(comb, B3SELF, inv3))
                red = red_tile()

                def ev_n3(mc, pst, red=red, last=last):
                    e2 = inv_mul(pst, inv2)
                    e = epool.tile([P, 2, RH], F32, tag="e")
                    nc.scalar.activation(
                        e[:], e2[:], mybir.ActivationFunctionType.Relu,
                        bias=bias_ap(B3PRE, mc))
                    nc.vector.scalar_tensor_tensor(
                        snew[:, mc, :], e[:], bias_ap(C3P, mc),
                        comb[:, mc, :],
                        op0=mybir.AluOpType.add, op1=mybir.AluOpType.add)
                    # s3(t2) is never consumed by a matmul: skip its fp8 copy
                    store_sq_reduce(mc, s3, red, copy=not last)

                term_pass("w3pre", KC, s2, ev_n3)
                finale(red, None if last else inv3, "add" if last else None)

            # ---- goodness out: g = gacc / (2048 * G^2)
            gout = consts.tile([1, R], F32, tag="gout")
            nc.scalar.mul(gout[:], gacc[:], 1.0 / (H * G * G))
            nc.sync.dma_start(out=g_d[:], in_=gout[:])

    nc.compile()
    return nc


def _block_weight(w, scale, kcn):
    """[2048, d_in] float32 -> [MC, P, kcn//2, 2, P] fp8 blocked and
    pair-interleaved for DoubleRowSwInterleave LDWEIGHTS: per k-pair the
    256 stationary columns are [A127, B127, A126, B126, ..., A0, B0]
    (A = even k-chunk, B = odd k-chunk, columns reversed)."""
    w = np.asarray(w, dtype=np.float32) * scale
    din = w.shape[1]
    if din < kcn * P:
        w = np.pad(w, ((0, 0), (0, kcn * P - din)))
    blk = w.reshape(MC, P, kcn, P).transpose(0, 3, 2, 1)
    blk = np.clip(blk, -240.0, 240.0).astype(NPF8)
    sw = np.empty((MC, P, kcn // 2, 2 * P), dtype=NPF8)
    sw[..., 0::2] = blk[:, :, 0::2, ::-1]
    sw[..., 1::2] = blk[:, :, 1::2, ::-1]
    return np.ascontiguousarray(sw.reshape(MC, P, kcn // 2, 2, P))


def _col(v):
    """[2048] -> [128, 16] (partition-major bias layout)."""
    return np.asarray(v, dtype=np.float32).reshape(MC, P).T


def prepare_inputs(inputs):
    """Host prep: overlay+normalize Hx, prescale/block weights, pack biases.
    Returns (shared_map, per_core_hx list)."""
    x = np.asarray(inputs["x"], dtype=np.float32)
    mx = x.max()
    base = x.copy()
    base[:, :NL] = 0.0
    hx = np.tile(base[None, :, :], (NL, 1, 1))
    for l in range(NL):
        hx[l, :, l] = mx
    hx = hx.reshape(ROWS, D_IN)
    n = np.linalg.norm(hx, axis=1, keepdims=True)
    hxn = (G / (n + EPS)) * hx
    hxn = np.pad(hxn, ((0, 0), (0, D_IN_PAD - D_IN)))

    per_core_hx = []
    for c in range(N_CORES):
        h = hxn[c * R:(c + 1) * R].T            # [1024, 640]
        h = h.reshape(KC1, P, R).transpose(1, 0, 2)
        per_core_hx.append(np.ascontiguousarray(
            np.clip(h, -240.0, 240.0).astype(NPF8)))

    wa = ALPHA
    shared = {
        "w1pre": _block_weight(inputs["w1_pre"], 0.7 * wa, KC1),
        "w1post": _block_weight(inputs["w1_post"], 0.7 * wa, KC),
        "w1self": _block_weight(inputs["w1_self"], 0.3 * wa, KC),
        "w2pre": _block_weight(inputs["w2_pre"], 0.7 * wa, KC),
        "w2post": _block_weight(inputs["w2_post"], 0.7 * wa, KC),
        "w2self": _block_weight(inputs["w2_self"], 0.3 * wa, KC),
        "w3pre": _block_weight(inputs["w3_pre"], 0.7 * wa, KC),
        "w3self": _block_weight(inputs["w3_self"], 0.3 * wa, KC),
    }

    relu = lambda a: np.maximum(np.asarray(a, dtype=np.float32), 0.0)

    cols = np.empty((P, NBIAS * MC), dtype=np.float32)
    vals = {
        B1PRE: G * 0.7 * np.asarray(inputs["b1_pre"], np.float32),
        B1POST: G * 0.7 * np.asarray(inputs["b1_post"], np.float32),
        B1SELF: G * 0.3 * np.asarray(inputs["b1_self"], np.float32),
        B2PRE: G * 0.7 * np.asarray(inputs["b2_pre"], np.float32),
        B2POST: G * 0.7 * np.asarray(inputs["b2_post"], np.float32),
        B2SELF: G * 0.3 * np.asarray(inputs["b2_self"], np.float32),
        B3PRE: G * 0.7 * np.asarray(inputs["b3_pre"], np.float32),
        B3SELF: G * 0.3 * np.asarray(inputs["b3_self"], np.float32),
        C1: G * (0.7 * relu(inputs["b1_post"]) + 0.3 * relu(inputs["b1_self"])),
        C2: G * (0.7 * relu(inputs["b2_post"]) + 0.3 * relu(inputs["b2_self"])),
        C3: G * (0.7 * relu(inputs["b3_post"]) + 0.3 * relu(inputs["b3_self"])),
        C3P: G * 0.7 * relu(inputs["b3_post"]),
    }
    for idx, v in vals.items():
        cols[:, idx * MC:(idx + 1) * MC] = _col(v)
    shared["biases"] = np.ascontiguousarray(cols)

    return shared, per_core_hx


def run(inputs, trace=False):
    shared, per_core_hx = prepare_inputs(inputs)
    if "nc" not in _NC_CACHE:
        _NC_CACHE["nc"] = _build_nc()
    nc = _NC_CACHE["nc"]
    in_maps = [dict(shared, hxn=per_core_hx[c]) for c in range(N_CORES)]
    res = run_bass_kernel_spmd(nc, in_maps, core_ids=list(range(N_CORES)),
                               trace=trace)
    g = np.concatenate([res.results[c]["g"][0] for c in range(N_CORES)])
    out = g.reshape(NL, B).T.astype(np.float32)
    return np.ascontiguousarray(out), res


def kernel(**inputs):
    out, _ = run(inputs, trace=False)
    return out


# revision 18
# speedup vs baseline: 1.9836x; 1.0032x over previous
"""Trainium2 Bass kernel for the 3-metalayer forward-forward style MLP.

Distribution: the (10 labels x 512 batch) grid flattens to 5120 independent
rows; each of the 8 cores processes 640 rows (pure data parallelism, weights
replicated, no collectives).

Device-side algorithm (per core, rows R=640):
  - matmul inputs (states, overlay input, weights) are fp8 e4m3; every
    linear term runs as DoubleRow matmuls (K=256 per instruction, 2 fp8
    weights per PE cell -> 2x MACs/cycle).
  - the working state snew is bf16 at gain G=64; its fp8 copy s_i (made by
    an ACT dtype-convert) feeds the matmuls. The row normalization is
    applied to the matmul OUTPUT (inv is per-row, so it commutes through
    the matmul): a DVE psum*inv multiply feeds the ACT relu+bias eviction.
    This removes the normalize->matmul serial dependency between passes.
    Norms/goodness come from the bf16 snew, so fp8 quantization noise does
    not bias the normalization (no coherent rho^2/2 shrink).
  - psum tiles are bank PAIRS [128, 2, 512] (one bank per row-half), so
    each eviction step is a single wide [128, 2x320] op - halving the
    per-op overhead on DVE/ACT and deepening the psum pipeline to 3 mc
    blocks.
  - weights carry gain ALPHA=1024 and the 0.7/0.3 metalayer blend (relu
    positive homogeneity); inv = (G/ALPHA)/(sqrt(ss)+G*eps) folds all
    gains; goodness = ss/(2048*G^2).
  - row L2 norms: square (DVE, fp8 out) + fp8 DoubleRow ones-matmul
    reduction over partition pairs (8 matmuls per 2048 features, M=128
    broadcasts the sum to every partition for free).
  - pass order puts the self-terms (older inv) before the post-terms
    (fresher inv) when initializing the comb accumulator, so evictions
    never wait on a just-computed inv; the t1/t2 n1 combine (pure DVE/ACT
    work) is wedged mc-by-mc into the following w2self pass so the PE
    never idles on it.
"""

import numpy as np
import ml_dtypes

import concourse.bass as bass
import concourse.tile as tile
from concourse import bacc, mybir
from concourse.bass_utils import run_bass_kernel_spmd

BF = mybir.dt.bfloat16
F32 = mybir.dt.float32
FP8 = mybir.dt.float8e4
NPBF = ml_dtypes.bfloat16
NPF8 = ml_dtypes.float8_e4m3
DR = mybir.MatmulPerfMode.DoubleRowSwInterleave

N_CORES = 8
P = 128
D_IN = 784
D_IN_PAD = 1024           # 8 * 128 (even k-chunk count for DoubleRow pairs)
KC1 = 8                   # k-chunks for the 784->2048 matmul (padded)
KC = 16                   # k-chunks for 2048-contraction matmuls
MC = 16                   # output-feature chunks (2048 / 128)
H = 2048
B = 512
NL = 10
ROWS = NL * B             # 5120
R = ROWS // N_CORES       # 640 rows per core
RH = 320                  # psum row-chunk (one bank per row-half)
BK = 512                  # f32 elems per psum bank
EPS = 1e-4

ALPHA = 1024.0            # weight fp8 gain
G = 64.0                  # state gain

# bias/const column indices inside the packed [128, 12*16] bias tensor
B1PRE, B1POST, B1SELF, B2PRE, B2POST, B2SELF, B3PRE, B3SELF, C1, C2, C3, C3P = range(12)
NBIAS = 12

_NC_CACHE = {}


def _build_nc():
    """Build the single-core Tile program (same NEFF for all 8 cores)."""
    nc = bacc.Bacc("TRN2", target_bir_lowering=False, debug=False,
                   num_devices=N_CORES)

    hx_d = nc.dram_tensor("hxn", [P, KC1, R], FP8, kind="ExternalInput")
    w_d = {
        "w1pre": nc.dram_tensor("w1pre", [MC, P, KC1 // 2, 2, P], FP8,
                                kind="ExternalInput"),
    }
    for name in ("w1post", "w1self", "w2pre", "w2post", "w2self", "w3pre", "w3self"):
        w_d[name] = nc.dram_tensor(name, [MC, P, KC // 2, 2, P], FP8,
                                   kind="ExternalInput")
    bias_d = nc.dram_tensor("biases", [P, NBIAS * MC], F32, kind="ExternalInput")
    g_d = nc.dram_tensor("g", [1, R], F32, kind="ExternalOutput")

    with tile.TileContext(nc) as tc:
        with (
            tc.tile_pool(name="consts", bufs=1) as consts,
            tc.tile_pool(name="states", bufs=1) as states,
            tc.tile_pool(name="wpool", bufs=12) as wpool,
            tc.tile_pool(name="epool", bufs=6) as epool,
            tc.tile_pool(name="sqpool", bufs=4) as sqpool,
            tc.tile_pool(name="small", bufs=2) as small,
            tc.tile_pool(name="mmps", bufs=3, space="PSUM") as mmps,
            tc.tile_pool(name="redps", bufs=1, space="PSUM") as redps,
        ):
            # startup order: first hx chunk + first weight block must land
            # before anything else so the PE starts within ~1.5us
            hx = states.tile([P, KC1, R], FP8, tag="hxn")
            nc.sync.dma_start(out=hx[:, 0:2, :], in_=hx_d[:, 0:2, :])
            bias_sb = consts.tile([P, NBIAS * MC], F32)
            w0 = wpool.tile([P, KC1 // 2, 2, P], FP8, tag="w", name="w1pre0")
            nc.sync.dma_start(out=w0[:], in_=w_d["w1pre"][0])
            nc.sync.dma_start(out=bias_sb[:], in_=bias_d[:])
            nc.sync.dma_start(out=hx[:, 2:KC1, :], in_=hx_d[:, 2:KC1, :])
            # [128, 2, 128] fp8 ones: M=128 DoubleRow ones-matmul reduces a
            # partition-pair AND broadcasts the row sum to every partition
            ones8 = consts.tile([P, 2, P], FP8)
            nc.vector.memset(ones8[:], 1.0)
            gacc = consts.tile([1, R], F32)

            # warm the PE HAM clock gate with matmuls on a junk tile
            # (contents irrelevant, psum never read): written by the
            # otherwise-idle GPSIMD engine so the PE starts right after its
            # boot sequence, and >3.4us of activity ramps the clock to
            # 2.4GHz before real work
            junk = consts.tile([P, P], FP8, tag="junk")
            nc.gpsimd.memset(junk[:], 1.0)
            warm_ps = mmps.tile([P, 2, BK], F32, tag="mm", name="warm_ps")
            for _ in range(48):
                nc.tensor.matmul(warm_ps[:, 0, :P], junk[:], junk[:],
                                 start=True, stop=True)
            At = states.tile([P, MC, R], BF, tag="A")
            snew = states.tile([P, MC, R], BF, tag="snew")
            s1 = states.tile([P, MC, R], FP8, tag="s1")
            s2 = states.tile([P, MC, R], FP8, tag="s2")
            s3 = states.tile([P, MC, R], FP8, tag="s3")
            comb = states.tile([P, MC, R], BF, tag="comb")
            inv1 = states.tile([P, 2, RH], F32, tag="inv1")
            inv2 = states.tile([P, 2, RH], F32, tag="inv2")
            inv3 = states.tile([P, 2, RH], F32, tag="inv3")

            _red_uid = [0]

            def red_tile():
                _red_uid[0] += 1
                return redps.tile([P, 2, BK], F32, tag="red",
                                  name=f"red{_red_uid[0]}")

            def bias_ap(idx, mc):
                col = idx * MC + mc
                return bias_sb[:, col:col + 1]

            def rsl(rh):
                return slice(rh * RH, (rh + 1) * RH)

            def term_pass(wname, kcn, src, evict, w0_tile=None, defer=2,
                          wedge=None):
                """One linear term: stream weight blocks, accumulate psum
                bank-pairs, hand each [128, 2, 320] pair to `evict(mc, pst)`.

                DoubleRow: each matmul consumes a k-pair (K=256); the two
                row-halves run back-to-back on the same stationary weights.

                Evictions are emitted `defer` mc blocks late: the eviction
                chain (DVE inv-mul -> ACT relu -> DVE combine) has ~2us of
                cross-engine latency, and emitting it inline makes the
                strict-FIFO PE queue stall. Deferring places it behind
                independent matmul work.

                `wedge(mc)`, if given, emits independent non-PE work after
                each mc block's matmuls (used to hide the n1 combine)."""
                wd = w_d[wname]
                kpn = kcn // 2
                pending = []
                for mc in range(MC):
                    if mc == 0 and w0_tile is not None:
                        wt = w0_tile
                    else:
                        wt = wpool.tile([P, kpn, 2, P], FP8, tag="w")
                        nc.sync.dma_start(out=wt[:], in_=wd[mc])
                    pst = mmps.tile([P, 2, BK], F32, tag="mm",
                                    name=f"mm_{wname}_{mc}")
                    for kp in range(kpn):
                        for rh in range(2):
                            nc.tensor.matmul(
                                pst[:, rh, 0:RH], wt[:, kp, :, :],
                                src[:, 2 * kp:2 * kp + 2, rsl(rh)],
                                start=(kp == 0), stop=(kp == kpn - 1),
                                perf_mode=DR)
                    if wedge is not None:
                        wedge(mc)
                    pending.append((mc, pst))
                    if len(pending) > defer:
                        evict(*pending.pop(0))
                while pending:
                    evict(*pending.pop(0))

            _sqt = [None]

            def store_sq_reduce(mc, s8, red, copy=True, sq_on_act=False):
                """snew chunk mc is complete: make the fp8 matmul copy,
                square (fp8), and on odd mc reduce the pair into the red
                psum with a DoubleRow ones-matmul."""
                if copy:
                    nc.scalar.copy(s8[:, mc, :], snew[:, mc, :])
                if mc % 2 == 0:
                    _sqt[0] = sqpool.tile([P, 2, R], FP8, tag="sq",
                                          name=f"sq{mc}")
                sqt = _sqt[0]
                if sq_on_act:
                    # final pass: DVE is the backlogged engine there, ACT is
                    # light; Square on ACT shortens the closing drain
                    nc.scalar.activation(sqt[:, mc % 2, :], snew[:, mc, :],
                                         mybir.ActivationFunctionType.Square)
                else:
                    nc.vector.tensor_mul(sqt[:, mc % 2, :], snew[:, mc, :],
                                         snew[:, mc, :])
                if mc % 2 == 1:
                    pair = mc // 2
                    for rh in range(2):
                        nc.tensor.matmul(red[:, rh, 0:RH], ones8[:],
                                         sqt[:, :, rsl(rh)],
                                         start=(pair == 0), stop=(pair == 7),
                                         perf_mode=DR)

            def finale(red, inv_t, goodness):
                """red holds ss = sum(snew^2) per row (gain G^2), already
                broadcast across all 128 partitions. inv_t (if wanted) gets
                (G/ALPHA)/(sqrt(ss)+G*eps), the per-row factor that turns
                a raw psum into the normalized-input result."""
                if goodness:
                    for rh in range(2):
                        if goodness == "init":
                            nc.vector.tensor_copy(gacc[:, rsl(rh)],
                                                  red[0:1, rh, 0:RH])
                        else:
                            nc.vector.tensor_add(gacc[:, rsl(rh)],
                                                 gacc[:, rsl(rh)],
                                                 red[0:1, rh, 0:RH])
                if inv_t is None:
                    return
                nr = small.tile([P, 2, RH], F32, tag="nr")
                nc.scalar.activation(nr[:], red[:, :, 0:RH],
                                     mybir.ActivationFunctionType.Sqrt,
                                     scale=(ALPHA / G) ** 2)
                nc.vector.tensor_scalar_add(nr[:], nr[:], ALPHA * EPS)
                nc.vector.reciprocal_approx_fast(out=inv_t[:], in_=nr[:])

            def inv_mul(pst, inv_t):
                e2 = epool.tile([P, 2, RH], F32, tag="e2")
                nc.vector.tensor_mul(e2[:], pst[:, :, 0:RH], inv_t[:])
                return e2

            def evict_to(dst, bidx, inv_t):
                def ev(mc, pst):
                    e2 = inv_mul(pst, inv_t)
                    nc.scalar.activation(
                        dst[:, mc, :], e2[:],
                        mybir.ActivationFunctionType.Relu,
                        bias=bias_ap(bidx, mc))
                return ev

            def evict_add_comb(bidx, inv_t):
                def ev(mc, pst):
                    e2 = inv_mul(pst, inv_t)
                    e = epool.tile([P, 2, RH], F32, tag="e")
                    nc.scalar.activation(
                        e[:], e2[:], mybir.ActivationFunctionType.Relu,
                        bias=bias_ap(bidx, mc))
                    nc.vector.tensor_add(comb[:, mc, :], e[:], comb[:, mc, :])
                return ev

            # ---- A = relu((hxn @ w1pre')/ALPHA + G*0.7*b1pre), cached for
            # all steps (hx is host-prenormalized: no inv needed).
            # t0-n1 (snew = A + c1) is fused into the same pass.
            red = red_tile()

            def ev_a(mc, pst, red=red):
                nc.scalar.activation(
                    At[:, mc, :], pst[:, :, 0:RH],
                    mybir.ActivationFunctionType.Relu,
                    bias=bias_ap(B1PRE, mc), scale=1.0 / ALPHA)
                nc.vector.tensor_scalar_add(
                    snew[:, mc, :], At[:, mc, :], bias_ap(C1, mc))
                store_sq_reduce(mc, s1, red)

            term_pass("w1pre", KC1, hx, ev_a, w0_tile=w0)
            finale(red, inv1, None)

            # ---- t0, n2 / n3: single pre-term + const.
            # t1-n1's self/post term passes are wedged between them: they
            # only need s1(t0)/s2(t0) and don't touch comb (the t0 updates
            # don't use it), so their matmuls fill t0's serial-chain tails.
            def ev_t0(red, s8, inv_t, cidx, bpre):
                def ev(mc, pst):
                    e2 = inv_mul(pst, inv_t)
                    e = epool.tile([P, 2, RH], F32, tag="e")
                    nc.scalar.activation(
                        e[:], e2[:], mybir.ActivationFunctionType.Relu,
                        bias=bias_ap(bpre, mc))
                    nc.vector.tensor_scalar_add(
                        snew[:, mc, :], e[:], bias_ap(cidx, mc))
                    store_sq_reduce(mc, s8, red)
                return ev

            red = red_tile()
            term_pass("w2pre", KC, s1, ev_t0(red, s2, inv1, C2, B2PRE))
            finale(red, inv2, None)

            # self-term first (its inv is a pass older), post-term second:
            # an eviction never waits on a just-finalized inv
            term_pass("w1self", KC, s1, evict_to(comb, B1SELF, inv1))
            term_pass("w1post", KC, s2, evict_add_comb(B1POST, inv2))

            red = red_tile()
            term_pass("w3pre", KC, s2, ev_t0(red, s3, inv2, C3, B3PRE))
            finale(red, inv3, None)

            def n1_wedge(red):
                # n1 = A + relu(s1@w1self'+b) + relu(s2@w1post'+b): comb is
                # complete, so each mc chunk is pure DVE/ACT work; wedging it
                # into the w2self pass keeps the PE streaming. Lag by 2 mc
                # blocks so the wedge's reduce-matmuls queue well behind
                # their DVE producers (no PE stall on the n1 chain).
                def unit(mc):
                    nc.vector.tensor_add(snew[:, mc, :], At[:, mc, :],
                                         comb[:, mc, :])
                    store_sq_reduce(mc, s1, red)

                def wg(mc):
                    if mc >= 2:
                        unit(mc - 2)
                    if mc == MC - 1:
                        unit(MC - 2)
                        unit(MC - 1)
                return wg

            # ---- t1 / t2
            for t in (1, 2):
                last = (t == 2)
                if t == 2:
                    term_pass("w1self", KC, s1, evict_to(comb, B1SELF, inv1))
                    term_pass("w1post", KC, s2, evict_add_comb(B1POST, inv2))

                # n2 = relu(s1new@w2pre') + relu(s2@w2self') + relu(s3@w2post')
                red_n1 = red_tile()
                term_pass("w2self", KC, s2, evict_to(comb, B2SELF, inv2),
                          wedge=n1_wedge(red_n1))
                finale(red_n1, inv1, "init" if last else None)
                term_pass("w2post", KC, s3, evict_add_comb(B2POST, inv3))
                red = red_tile()

                def ev_n2(mc, pst, red=red):
                    e2 = inv_mul(pst, inv1)
                    e = epool.tile([P, 2, RH], F32, tag="e")
                    nc.scalar.activation(
                        e[:], e2[:], mybir.ActivationFunctionType.Relu,
                        bias=bias_ap(B2PRE, mc))
                    nc.vector.tensor_add(snew[:, mc, :], e[:], comb[:, mc, :])
                    store_sq_reduce(mc, s2, red)

                term_pass("w2pre", KC, s1, ev_n2)
                finale(red, inv2, "add" if last else None)

                # n3 = relu(s2new@w3pre') + c3p + relu(s3@w3self')
                term_pass("w3self", KC, s3, evict_to(comb, B3SELF, inv3))
                red = red_tile()

                def ev_n3(mc, pst, red=red, last=last):
                    e2 = inv_mul(pst, inv2)
                    e = epool.tile([P, 2, RH], F32, tag="e")
                    nc.scalar.activation(
                        e[:], e2[:], mybir.ActivationFunctionType.Relu,
                        bias=bias_ap(B3PRE, mc))
                    nc.vector.scalar_tensor_tensor(
                        snew[:, mc, :], e[:], bias_ap(C3P, mc),
                        comb[:, mc, :],
                        op0=mybir.AluOpType.add, op1=mybir.AluOpType.add)
                    # s3(t2) is never consumed by a matmul: skip its fp8 copy
                    store_sq_reduce(mc, s3, red, copy=not last,
                                    sq_on_act=last)

                term_pass("w3pre", KC, s2, ev_n3, defer=1 if last else 2)
                finale(red, None if last else inv3, "add" if last else None)

            # ---- goodness out: g = gacc / (2048 * G^2)
            gout = consts.tile([1, R], F32, tag="gout")
            nc.scalar.mul(gout[:], gacc[:], 1.0 / (H * G * G))
            nc.sync.dma_start(out=g_d[:], in_=gout[:])

    nc.compile()
    return nc


def _block_weight(w, scale, kcn):
    """[2048, d_in] float32 -> [MC, P, kcn//2, 2, P] fp8 blocked and
    pair-interleaved for DoubleRowSwInterleave LDWEIGHTS: per k-pair the
    256 stationary columns are [A127, B127, A126, B126, ..., A0, B0]
    (A = even k-chunk, B = odd k-chunk, columns reversed)."""
    w = np.asarray(w, dtype=np.float32) * scale
    din = w.shape[1]
    if din < kcn * P:
        w = np.pad(w, ((0, 0), (0, kcn * P - din)))
    blk = w.reshape(MC, P, kcn, P).transpose(0, 3, 2, 1)
    blk = np.clip(blk, -240.0, 240.0).astype(NPF8)
    sw = np.empty((MC, P, kcn // 2, 2 * P), dtype=NPF8)
    sw[..., 0::2] = blk[:, :, 0::2, ::-1]
    sw[..., 1::2] = blk[:, :, 1::2, ::-1]
    return np.ascontiguousarray(sw.reshape(MC, P, kcn // 2, 2, P))


def _col(v):
    """[2048] -> [128, 16] (partition-major bias layout)."""
    return np.asarray(v, dtype=np.float32).reshape(MC, P).T


def prepare_inputs(inputs):
    """Host prep: overlay+normalize Hx, prescale/block weights, pack biases.
    Returns (shared_map, per_core_hx list)."""
    x = np.asarray(inputs["x"], dtype=np.float32)
    mx = x.max()
    base = x.copy()
    base[:, :NL] = 0.0
    hx = np.tile(base[None, :, :], (NL, 1, 1))
    for l in range(NL):
        hx[l, :, l] = mx
    hx = hx.reshape(ROWS, D_IN)
    n = np.linalg.norm(hx, axis=1, keepdims=True)
    hxn = (G / (n + EPS)) * hx
    hxn = np.pad(hxn, ((0, 0), (0, D_IN_PAD - D_IN)))

    per_core_hx = []
    for c in range(N_CORES):
        h = hxn[c * R:(c + 1) * R].T            # [1024, 640]
        h = h.reshape(KC1, P, R).transpose(1, 0, 2)
        per_core_hx.append(np.ascontiguousarray(
            np.clip(h, -240.0, 240.0).astype(NPF8)))

    wa = ALPHA
    shared = {
        "w1pre": _block_weight(inputs["w1_pre"], 0.7 * wa, KC1),
        "w1post": _block_weight(inputs["w1_post"], 0.7 * wa, KC),
        "w1self": _block_weight(inputs["w1_self"], 0.3 * wa, KC),
        "w2pre": _block_weight(inputs["w2_pre"], 0.7 * wa, KC),
        "w2post": _block_weight(inputs["w2_post"], 0.7 * wa, KC),
        "w2self": _block_weight(inputs["w2_self"], 0.3 * wa, KC),
        "w3pre": _block_weight(inputs["w3_pre"], 0.7 * wa, KC),
        "w3self": _block_weight(inputs["w3_self"], 0.3 * wa, KC),
    }

    relu = lambda a: np.maximum(np.asarray(a, dtype=np.float32), 0.0)

    cols = np.empty((P, NBIAS * MC), dtype=np.float32)
    vals = {
        B1PRE: G * 0.7 * np.asarray(inputs["b1_pre"], np.float32),
        B1POST: G * 0.7 * np.asarray(inputs["b1_post"], np.float32),
        B1SELF: G * 0.3 * np.asarray(inputs["b1_self"], np.float32),
        B2PRE: G * 0.7 * np.asarray(inputs["b2_pre"], np.float32),
        B2POST: G * 0.7 * np.asarray(inputs["b2_post"], np.float32),
        B2SELF: G * 0.3 * np.asarray(inputs["b2_self"], np.float32),
        B3PRE: G * 0.7 * np.asarray(inputs["b3_pre"], np.float32),
        B3SELF: G * 0.3 * np.asarray(inputs["b3_self"], np.float32),
        C1: G * (0.7 * relu(inputs["b1_post"]) + 0.3 * relu(inputs["b1_self"])),
        C2: G * (0.7 * relu(inputs["b2_post"]) + 0.3 * relu(inputs["b2_self"])),
        C3: G * (0.7 * relu(inputs["b3_post"]) + 0.3 * relu(inputs["b3_self"])),
        C3P: G * 0.7 * relu(inputs["b3_post"]),
    }
    for idx, v in vals.items():
        cols[:, idx * MC:(idx + 1) * MC] = _col(v)
    shared["biases"] = np.ascontiguousarray(cols)

    return shared, per_core_hx


def run(inputs, trace=False):
    shared, per_core_hx = prepare_inputs(inputs)
    if "nc" not in _NC_CACHE:
        _NC_CACHE["nc"] = _build_nc()
    nc = _NC_CACHE["nc"]
    in_maps = [dict(shared, hxn=per_core_hx[c]) for c in range(N_CORES)]
    res = run_bass_kernel_spmd(nc, in_maps, core_ids=list(range(N_CORES)),
                               trace=trace)
    g = np.concatenate([res.results[c]["g"][0] for c in range(N_CORES)])
    out = g.reshape(NL, B).T.astype(np.float32)
    return np.ascontiguousarray(out), res


def kernel(**inputs):
    out, _ = run(inputs, trace=False)
    return out
